# revision 1
# baseline (speedup 1.0000x reference)
"""DAGNN recommender forward pass on 8 Trainium2 NeuronCores (Bass/Tile).

Strategy (nodes sharded across cores, dst-block scatter via selection-matrix
matmuls, per-hop AllGather of the updated node states):

 - Nodes are relabeled by a host-side permutation so each 128-row "block"
   has a balanced in-edge count (<= 2048).  400 blocks total, 50 per core.
 - Per hop, each core gathers cur[src] rows for its ~100k in-edges with
   `dma_gather` (512B rows from a replicated DRAM copy of cur), then for
   every dst block accumulates 16 matmuls  psum += S_chunk.T @ G_chunk
   where S is the one-hot (edge -> dst row) selection matrix built on the
   fly with iota + is_equal.  Gather indices are int16, so gathers read
   from two 32768-row windows of cur (A = rows [0, 32768), B = last 32768
   rows); the host assigns each edge to a window.
 - softmax(att) hop weights are folded into cur (cur'_i = w_i A^i h), so
   the DAGNN output is just the running sum of cur' slices.
 - The input/output MLPs run feature-major (activations transposed) so
   BatchNorm's per-feature statistics live on partitions; train-mode BN
   makes the b1/b2/b3 biases cancel, and the zero-padded fake nodes are
   corrected analytically in the BN statistics.
 - Cross-core traffic: one AllGather of the cur slices per hop plus three
   tiny AllReduces for BN statistics.
"""

import os
import sys

if "/opt/trn_rl_repo" not in sys.path:
    sys.path.insert(0, "/opt/trn_rl_repo")

import numpy as np


# ---------------------------------------------------------------- config

class Cfg:
    def __init__(self, N=50000, E=800000, K=10, BPC=50, OUT=1000):
        self.N, self.E, self.K, self.BPC, self.OUT = N, E, K, BPC, OUT
        self.H = 128
        self.IN = 128
        self.NCORES = 8
        self.BLK = 128
        self.NBLOCKS = self.NCORES * BPC
        self.NP = self.NBLOCKS * self.BLK
        self.ROWS = BPC * self.BLK              # rows per core
        self.ACH = 8
        self.BCH = 8
        self.CH = self.ACH + self.BCH
        self.ASLOTS = self.ACH * self.BLK       # 1024
        self.BSLOTS = self.BCH * self.BLK
        self.WINW = min(32768, self.NP)
        self.WINA_LO = 0
        self.WINB_LO = self.NP - self.WINW
        self.NPADN = self.NP - N
        self.GG = 2                              # blocks per gather group
        assert BPC % self.GG == 0
        self.NGROUPS = BPC // self.GG
        self.EPS = 1e-5


FULL = Cfg()


# ---------------------------------------------------------------- host prep

def balance_nodes(cfg: Cfg, dst: np.ndarray) -> np.ndarray:
    """perm: old node id (incl. pads) -> new padded row id, balancing
    per-block in-edge counts (LPT greedy)."""
    import heapq
    deg = np.bincount(dst, minlength=cfg.N).astype(np.int64)
    deg_all = np.concatenate([deg, np.zeros(cfg.NP - cfg.N, np.int64)])
    order = np.argsort(-deg_all, kind="stable")
    load = np.zeros(cfg.NBLOCKS, np.int64)
    fill = np.zeros(cfg.NBLOCKS, np.int64)
    perm = np.empty(cfg.NP, np.int64)
    heap = [(0, b) for b in range(cfg.NBLOCKS)]
    heapq.heapify(heap)
    for node in order:
        while True:
            _, b = heapq.heappop(heap)
            if fill[b] < cfg.BLK:
                break
        perm[node] = b * cfg.BLK + fill[b]
        fill[b] += 1
        load[b] += deg_all[node]
        if fill[b] < cfg.BLK:
            heapq.heappush(heap, (load[b], b))
    return perm


def build_edge_structures(cfg: Cfg, edge_index: np.ndarray):
    """(perm, idx_img [8,128,BPC*128] int16, dst_img [8,128,BPC*16] f32)."""
    src, dst = edge_index[0].astype(np.int64), edge_index[1].astype(np.int64)
    perm = balance_nodes(cfg, dst)
    psrc = perm[src]
    pdst = perm[dst]
    blk = pdst // cfg.BLK
    rel = pdst % cfg.BLK
    order = np.argsort(blk, kind="stable")
    psrc, rel, blk = psrc[order], rel[order], blk[order]
    starts = np.searchsorted(blk, np.arange(cfg.NBLOCKS + 1))

    acols = cfg.ASLOTS // 16                    # idx cols per block per region
    idx_img = np.zeros((cfg.NCORES, 16, 2 * cfg.BPC * acols), np.int16)
    dst_img = np.full((cfg.NCORES, 128, cfg.BPC * cfg.CH), -1.0, np.float32)
    aoff = cfg.BPC * acols                      # col offset of B region

    for b in range(cfg.NBLOCKS):
        s, e = starts[b], starts[b + 1]
        bs, br = psrc[s:e], rel[s:e]
        assert e - s <= cfg.ASLOTS + cfg.BSLOTS, (b, e - s)
        mustA = bs < cfg.WINB_LO
        mustB = bs >= cfg.WINW
        flex = ~mustA & ~mustB
        nA, nB, nF = int(mustA.sum()), int(mustB.sum()), int(flex.sum())
        assert nA <= cfg.ASLOTS and nB <= cfg.BSLOTS, (b, nA, nB)
        takeA = min(cfg.ASLOTS - nA, nF)
        assert nB + (nF - takeA) <= cfg.BSLOTS, (b, nA, nB, nF)
        fi = np.nonzero(flex)[0]
        a_sel = np.concatenate([np.nonzero(mustA)[0], fi[:takeA]])
        b_sel = np.concatenate([np.nonzero(mustB)[0], fi[takeA:]])

        a_idx = np.zeros(cfg.ASLOTS, np.int64)
        a_rel = np.full(cfg.ASLOTS, -1.0, np.float32)
        a_idx[:len(a_sel)] = bs[a_sel] - cfg.WINA_LO
        a_rel[:len(a_sel)] = br[a_sel]
        b_idx = np.zeros(cfg.BSLOTS, np.int64)
        b_rel = np.full(cfg.BSLOTS, -1.0, np.float32)
        b_idx[:len(b_sel)] = bs[b_sel] - cfg.WINB_LO
        b_rel[:len(b_sel)] = br[b_sel]
        assert 0 <= a_idx.min(initial=0) and a_idx.max(initial=0) < 32768
        assert 0 <= b_idx.min(initial=0) and b_idx.max(initial=0) < 32768

        c, lb = divmod(b, cfg.BPC)
        idx_img[c, :, lb * acols:(lb + 1) * acols] = \
            a_idx.reshape(acols, 16).T.astype(np.int16)
        idx_img[c, :, aoff + lb * acols: aoff + (lb + 1) * acols] = \
            b_idx.reshape(acols, 16).T.astype(np.int16)
        dst_img[c, :, lb * cfg.ACH:(lb + 1) * cfg.ACH] = \
            a_rel.reshape(cfg.ACH, 128).T
        boff = cfg.BPC * cfg.ACH
        dst_img[c, :, boff + lb * cfg.BCH: boff + (lb + 1) * cfg.BCH] = \
            b_rel.reshape(cfg.BCH, 128).T

    idx_img = np.tile(idx_img, (1, 8, 1))       # replicate to 128 partitions
    return perm, idx_img, dst_img


# ---------------------------------------------------------------- device

def build_nc(cfg: Cfg):
    from concourse import bass, mybir, bacc, tile

    F32 = mybir.dt.float32
    I16 = mybir.dt.int16
    I32 = mybir.dt.int32
    AF = mybir.ActivationFunctionType
    OP = mybir.AluOpType
    BLK, BPC, CH, ACH, BCH = cfg.BLK, cfg.BPC, cfg.CH, cfg.ACH, cfg.BCH
    ROWS, NP, K, H, OUT = cfg.ROWS, cfg.NP, cfg.K, cfg.H, cfg.OUT
    GG, NG = cfg.GG, cfg.NGROUPS
    acols = cfg.ASLOTS // 16
    NCORES = cfg.NCORES
    invN = 1.0 / cfg.N
    OUT_A = min(512, OUT)
    OUT_B = OUT - OUT_A

    nc = bacc.Bacc("TRN2", target_bir_lowering=False, debug=False,
                   num_devices=NCORES)

    x_sh = nc.dram_tensor("x_sh", [ROWS, cfg.IN], F32, kind="ExternalInput")
    idxs = nc.dram_tensor("idxs", [128, 2 * BPC * acols], I16, kind="ExternalInput")
    drel = nc.dram_tensor("drel", [128, BPC * CH], F32, kind="ExternalInput")
    W1 = nc.dram_tensor("W1", [cfg.IN, H], F32, kind="ExternalInput")
    W2 = nc.dram_tensor("W2", [H, H], F32, kind="ExternalInput")
    W3 = nc.dram_tensor("W3", [H, H // 2], F32, kind="ExternalInput")
    Wout = nc.dram_tensor("Wout", [H // 2, OUT], F32, kind="ExternalInput")
    g1 = nc.dram_tensor("g1", [H, 1], F32, kind="ExternalInput")
    be1 = nc.dram_tensor("be1", [H, 1], F32, kind="ExternalInput")
    g2 = nc.dram_tensor("g2", [H, 1], F32, kind="ExternalInput")
    be2 = nc.dram_tensor("be2", [H, 1], F32, kind="ExternalInput")
    g3 = nc.dram_tensor("g3", [H // 2, 1], F32, kind="ExternalInput")
    be3 = nc.dram_tensor("be3", [H // 2, 1], F32, kind="ExternalInput")
    att = nc.dram_tensor("att", [1, K + 1], F32, kind="ExternalInput")
    bout = nc.dram_tensor("bout", [1, OUT], F32, kind="ExternalInput")
    out = nc.dram_tensor("out", [ROWS, OUT], F32, kind="ExternalOutput")

    rg = [list(range(NCORES))]

    with tile.TileContext(nc) as tc:
        with (
            tc.tile_pool(name="const", bufs=1) as cpool,
            tc.tile_pool(name="resid", bufs=1) as rpool,
            tc.tile_pool(name="dram", bufs=1, space="DRAM") as dpool,
        ):
            # ---------------- persistent DRAM buffers
            cur0 = dpool.tile([NP, H], F32, tag="cur0")
            cur1 = dpool.tile([NP, H], F32, tag="cur1")
            ag_in = dpool.tile([ROWS, H], F32, tag="ag_in")
            bn_in_d = [dpool.tile([H, 2], F32, tag=f"bni{i}", name=f"bni{i}") for i in range(3)]
            bn_out_d = [dpool.tile([H, 2], F32, tag=f"bno{i}", name=f"bno{i}") for i in range(3)]

            # ---------------- constants / weights to SBUF
            w1sb = cpool.tile([cfg.IN, H], F32)
            nc.sync.dma_start(w1sb[:], W1[:])
            w2sb = cpool.tile([H, H], F32)
            nc.sync.dma_start(w2sb[:], W2[:])
            w3sb = cpool.tile([H, H // 2], F32)
            nc.sync.dma_start(w3sb[:], W3[:])
            wosb = cpool.tile([H // 2, OUT], F32)
            nc.sync.dma_start(wosb[:], Wout[:])
            g1sb = cpool.tile([H, 1], F32); nc.sync.dma_start(g1sb[:], g1[:])
            be1sb = cpool.tile([H, 1], F32); nc.sync.dma_start(be1sb[:], be1[:])
            g2sb = cpool.tile([H, 1], F32); nc.sync.dma_start(g2sb[:], g2[:])
            be2sb = cpool.tile([H, 1], F32); nc.sync.dma_start(be2sb[:], be2[:])
            g3sb = cpool.tile([H // 2, 1], F32); nc.sync.dma_start(g3sb[:], g3[:])
            be3sb = cpool.tile([H // 2, 1], F32); nc.sync.dma_start(be3sb[:], be3[:])
            attsb = cpool.tile([1, K + 1], F32); nc.sync.dma_start(attsb[:], att[:])
            bosb = cpool.tile([1, OUT], F32); nc.sync.dma_start(bosb[:], bout[:])

            idx_sb = cpool.tile([128, 2 * BPC * acols], I16)
            nc.sync.dma_start(idx_sb[:], idxs[:])
            dstT = cpool.tile([128, BPC * CH], F32)
            nc.sync.dma_start(dstT[:], drel[:])

            identity = cpool.tile([128, 128], F32)
            from concourse.masks import make_identity
            make_identity(nc, identity[:])

            iota_i = cpool.tile([128, ACH * 128], I32)
            nc.gpsimd.iota(iota_i[:].rearrange("p (c d) -> p c d", d=128),
                           pattern=[[0, ACH], [1, 128]], base=0,
                           channel_multiplier=0)
            iota8f = cpool.tile([128, ACH * 128], F32)
            nc.vector.tensor_copy(iota8f[:], iota_i[:])

            ones1 = cpool.tile([1, 128], F32)
            nc.vector.memset(ones1[:], 1.0)
            epsc = cpool.tile([128, 1], F32)
            nc.vector.memset(epsc[:], cfg.EPS)

            # softmax(att) -> w[0..K], ratios r_i = w_i/w_{i-1} -> wR [128, 2K+1]
            mx = cpool.tile([1, 1], F32)
            nc.vector.tensor_reduce(mx[:], attsb[:], axis=mybir.AxisListType.X,
                                    op=OP.max)
            nmx = cpool.tile([1, 1], F32)
            nc.scalar.mul(nmx[:], mx[:], -1.0)
            ew = cpool.tile([1, K + 1], F32)
            nc.scalar.activation(ew[:], attsb[:], AF.Exp, bias=nmx[:, 0:1],
                                 scale=1.0)
            ssum = cpool.tile([1, 1], F32)
            nc.vector.tensor_reduce(ssum[:], ew[:], axis=mybir.AxisListType.X,
                                    op=OP.add)
            rsum = cpool.tile([1, 1], F32)
            nc.vector.reciprocal(rsum[:], ssum[:])
            wv = cpool.tile([1, K + 1], F32)
            nc.vector.tensor_scalar_mul(wv[:], ew[:], rsum[:, 0:1])
            rw = cpool.tile([1, K + 1], F32)
            nc.vector.reciprocal(rw[:], wv[:])
            wcat = cpool.tile([1, 2 * K + 1], F32)
            nc.vector.tensor_copy(wcat[:, 0:K + 1], wv[:])
            nc.vector.tensor_tensor(out=wcat[:, K + 1:2 * K + 1],
                                    in0=wv[:, 1:K + 1], in1=rw[:, 0:K],
                                    op=OP.mult)
            with tc.tile_pool(name="wps", bufs=1, space="PSUM") as wps:
                wpsum = wps.tile([128, 2 * K + 1], F32, space="PSUM")
                nc.tensor.matmul(out=wpsum[:], lhsT=ones1[:], rhs=wcat[:],
                                 start=True, stop=True)
                wR = cpool.tile([128, 2 * K + 1], F32)
                nc.scalar.copy(wR[:], wpsum[:])

                # bout replicated to 128 partitions
                boutR = cpool.tile([128, OUT], F32)
                bps_a = wps.tile([128, OUT_A], F32, space="PSUM", tag="bps")
                nc.tensor.matmul(out=bps_a[:], lhsT=ones1[:],
                                 rhs=bosb[:, 0:OUT_A], start=True, stop=True)
                nc.scalar.copy(boutR[:, 0:OUT_A], bps_a[:])
                if OUT_B:
                    bps_b = wps.tile([128, OUT_B], F32, space="PSUM", tag="bps")
                    nc.tensor.matmul(out=bps_b[:], lhsT=ones1[:],
                                     rhs=bosb[:, OUT_A:OUT], start=True,
                                     stop=True)
                    nc.scalar.copy(boutR[:, OUT_A:OUT], bps_b[:])

            # ---------------- resident activations
            stage = rpool.tile([128, ROWS], F32, tag="stage")    # node-major cur'
            accum = rpool.tile([128, ROWS], F32, tag="accum")    # sum of cur'

            # ================ input MLP (feature-major) ================
            with (
                tc.tile_pool(name="mlp_a", bufs=1) as apool,
                tc.tile_pool(name="mlp_t", bufs=4) as tpool,
                tc.tile_pool(name="mlp_ps", bufs=3, space="PSUM") as mpps,
                tc.tile_pool(name="stat", bufs=1) as spool,
            ):
                a1 = apool.tile([128, ROWS], F32, tag="a1")      # z1T then h1T
                a2 = apool.tile([128, ROWS], F32, tag="a2")      # z2T then h2T
                scol = spool.tile([128, BPC], F32, tag="scol")
                qcol = spool.tile([128, BPC], F32, tag="qcol")
                bn_sb = [spool.tile([128, 2], F32, tag=f"bnsb{i}", name=f"bnsb{i}")
                         for i in range(3)]
                bnst = [spool.tile([128, 6], F32, tag=f"bnst{i}", name=f"bnst{i}")
                        for i in range(3)]  # mean, ex2, var, std, scale, shift

                def bn_stats_finish(i, gsb, besb, parts=128):
                    """bn_sb[i][:,0]=sum, [:,1]=sumsq (already AllReduced &
                    pad-corrected) -> bnst[i] cols: scale(4), shift(5)."""
                    st = bnst[i]
                    nc.scalar.mul(st[:parts, 0:1], bn_sb[i][:parts, 0:1], invN)
                    nc.scalar.mul(st[:parts, 1:2], bn_sb[i][:parts, 1:2], invN)
                    nc.vector.tensor_tensor(out=st[:parts, 2:3],
                                            in0=st[:parts, 0:1],
                                            in1=st[:parts, 0:1], op=OP.mult)
                    nc.vector.tensor_tensor(out=st[:parts, 2:3],
                                            in0=st[:parts, 1:2],
                                            in1=st[:parts, 2:3], op=OP.subtract)
                    nc.scalar.activation(st[:parts, 3:4], st[:parts, 2:3],
                                         AF.Sqrt, bias=epsc[:parts, 0:1],
                                         scale=1.0)
                    nc.vector.reciprocal(st[:parts, 4:5], st[:parts, 3:4])
                    nc.vector.tensor_tensor(out=st[:parts, 4:5],
                                            in0=st[:parts, 4:5],
                                            in1=gsb[:parts, 0:1], op=OP.mult)
                    nc.vector.tensor_tensor(out=st[:parts, 5:6],
                                            in0=st[:parts, 0:1],
                                            in1=st[:parts, 4:5], op=OP.mult)
                    nc.vector.tensor_tensor(out=st[:parts, 5:6],
                                            in0=besb[:parts, 0:1],
                                            in1=st[:parts, 5:6], op=OP.subtract)

                def bn_allreduce(i, parts=128):
                    nc.sync.dma_start(bn_in_d[i][:parts, :], bn_sb[i][:parts, :])
                    if parts < 128:
                        zf = spool.tile([128 - parts, 2], F32, tag="zfill")
                        nc.vector.memset(zf[:], 0.0)
                        nc.sync.dma_start(bn_in_d[i][parts:, :], zf[:])
                    nc.gpsimd.collective_compute(
                        "AllReduce", OP.add, replica_groups=rg,
                        ins=[bn_in_d[i][:].opt()], outs=[bn_out_d[i][:].opt()])
                    nc.sync.dma_start(bn_sb[i][:parts, :], bn_out_d[i][:parts, :])

                # ---- MLP1: z1T = W1.T @ xT
                for t in range(BPC):
                    xe = tpool.tile([128, 128], F32, tag="xload")
                    nc.sync.dma_start(xe[:], x_sh[t * BLK:(t + 1) * BLK, :])
                    xtp = mpps.tile([128, 128], F32, space="PSUM", tag="xtp")
                    nc.tensor.transpose(xtp[:], xe[:], identity[:])
                    xt = tpool.tile([128, 128], F32, tag="xt")
                    nc.scalar.copy(xt[:], xtp[:])
                    zp = mpps.tile([128, 128], F32, space="PSUM", tag="zp")
                    nc.tensor.matmul(out=zp[:], lhsT=w1sb[:], rhs=xt[:],
                                     start=True, stop=True)
                    tcols = slice(t * BLK, (t + 1) * BLK)
                    nc.scalar.copy(a1[:, tcols], zp[:])
                    nc.vector.tensor_reduce(scol[:, t:t + 1], a1[:, tcols],
                                            axis=mybir.AxisListType.X, op=OP.add)
                    sq = tpool.tile([128, 128], F32, tag="sq")
                    nc.scalar.square(sq[:], a1[:, tcols])
                    nc.vector.tensor_reduce(qcol[:, t:t + 1], sq[:],
                                            axis=mybir.AxisListType.X, op=OP.add)
                nc.vector.tensor_reduce(bn_sb[0][:, 0:1], scol[:],
                                        axis=mybir.AxisListType.X, op=OP.add)
                nc.vector.tensor_reduce(bn_sb[0][:, 1:2], qcol[:],
                                        axis=mybir.AxisListType.X, op=OP.add)
                bn_allreduce(0)
                bn_stats_finish(0, g1sb, be1sb)
                sc1 = bnst[0][:, 4:5]
                sh1 = bnst[0][:, 5:6]
                # h1T = relu(sc1 * z1T + sh1), in place
                for t in range(BPC):
                    tcols = slice(t * BLK, (t + 1) * BLK)
                    nc.scalar.activation(a1[:, tcols], a1[:, tcols], AF.Relu,
                                         bias=sh1, scale=sc1)

                # pad row value hpad1 = relu(sh1)
                hpad1 = spool.tile([128, 1], F32, tag="hpad1")
                nc.scalar.activation(hpad1[:], sh1, AF.Relu)
                zpad2 = spool.tile([128, 1], F32, tag="zpad2")
                zp2p = mpps.tile([128, 1], F32, space="PSUM", tag="zp")
                nc.tensor.matmul(out=zp2p[:], lhsT=w2sb[:], rhs=hpad1[:],
                                 start=True, stop=True)
                nc.scalar.copy(zpad2[:], zp2p[:])

                # ---- MLP2: z2T = W2.T @ h1T
                for t in range(BPC):
                    tcols = slice(t * BLK, (t + 1) * BLK)
                    zp = mpps.tile([128, 128], F32, space="PSUM", tag="zp")
                    nc.tensor.matmul(out=zp[:], lhsT=w2sb[:], rhs=a1[:, tcols],
                                     start=True, stop=True)
                    nc.scalar.copy(a2[:, tcols], zp[:])
                    nc.vector.tensor_reduce(scol[:, t:t + 1], a2[:, tcols],
                                            axis=mybir.AxisListType.X, op=OP.add)
                    sq = tpool.tile([128, 128], F32, tag="sq")
                    nc.scalar.square(sq[:], a2[:, tcols])
                    nc.vector.tensor_reduce(qcol[:, t:t + 1], sq[:],
                                            axis=mybir.AxisListType.X, op=OP.add)
                nc.vector.tensor_reduce(bn_sb[1][:, 0:1], scol[:],
                                        axis=mybir.AxisListType.X, op=OP.add)
                nc.vector.tensor_reduce(bn_sb[1][:, 1:2], qcol[:],
                                        axis=mybir.AxisListType.X, op=OP.add)
                # pad correction (only core 0 contributes it, pre-AllReduce)
                # every core computes sum over its own rows; pads exist on all
                # cores' slices, already included -> subtract npad * zpad2
                # AFTER AllReduce.  Simpler: subtract (npad/NCORES-weighted)?
                # pads per core vary; correct GLOBALLY after the AllReduce.
                bn_allreduce(1)
                corr = spool.tile([128, 2], F32, tag="corr")
                nc.scalar.mul(corr[:, 0:1], zpad2[:], -float(cfg.NPADN))
                sqz = spool.tile([128, 1], F32, tag="sqz")
                nc.scalar.square(sqz[:], zpad2[:])
                nc.scalar.mul(corr[:, 1:2], sqz[:], -float(cfg.NPADN))
                nc.vector.tensor_tensor(out=bn_sb[1][:], in0=bn_sb[1][:],
                                        in1=corr[:], op=OP.add)
                bn_stats_finish(1, g2sb, be2sb)
                sc2 = bnst[1][:, 4:5]
                sh2 = bnst[1][:, 5:6]
                # h2T = relu(sc2*z2T + sh2) + h1T   (a2 in place)
                for t in range(BPC):
                    tcols = slice(t * BLK, (t + 1) * BLK)
                    nc.scalar.activation(a2[:, tcols], a2[:, tcols], AF.Relu,
                                         bias=sh2, scale=sc2)
                    nc.vector.tensor_tensor(out=a2[:, tcols], in0=a2[:, tcols],
                                            in1=a1[:, tcols], op=OP.add)

                # hpad2 = relu(sc2*zpad2+sh2) + hpad1 ; zpad3 = (w0*hpad2)@W3
                hpad2 = spool.tile([128, 1], F32, tag="hpad2")
                nc.scalar.activation(hpad2[:], zpad2[:], AF.Relu, bias=sh2,
                                     scale=sc2)
                nc.vector.tensor_tensor(out=hpad2[:], in0=hpad2[:],
                                        in1=hpad1[:], op=OP.add)
                w0hpad2 = spool.tile([128, 1], F32, tag="w0hpad2")
                nc.scalar.activation(w0hpad2[:], hpad2[:], AF.Copy, bias=0.0,
                                     scale=wR[:, 0:1])
                zpad3 = spool.tile([64, 1], F32, tag="zpad3")
                zp3p = mpps.tile([64, 1], F32, space="PSUM", tag="zp")
                nc.tensor.matmul(out=zp3p[:], lhsT=w3sb[:], rhs=w0hpad2[:],
                                 start=True, stop=True)
                nc.scalar.copy(zpad3[:], zp3p[:])
                zpad3_keep = rpool.tile([64, 2], F32, tag="zpad3k")
                nc.scalar.mul(zpad3_keep[:, 0:1], zpad3[:], -float(cfg.NPADN))
                sq3 = spool.tile([64, 1], F32, tag="sq3")
                nc.scalar.square(sq3[:], zpad3[:])
                nc.scalar.mul(zpad3_keep[:, 1:2], sq3[:], -float(cfg.NPADN))

                # stage = h2 node-major; accum = w0 * stage
                for t in range(BPC):
                    tcols = slice(t * BLK, (t + 1) * BLK)
                    htp = mpps.tile([128, 128], F32, space="PSUM", tag="xtp")
                    nc.tensor.transpose(htp[:], a2[:, tcols], identity[:])
                    nc.scalar.copy(stage[:, tcols], htp[:])
                    nc.scalar.activation(accum[:, tcols], stage[:, tcols],
                                         AF.Copy, bias=0.0, scale=wR[:, 0:1])

            # write cur'_0 and AllGather it
            nc.sync.dma_start(
                ag_in[:].rearrange("(b p) f -> p b f", p=BLK),
                stage[:].rearrange("p (b f) -> p b f", f=H))
            nc.gpsimd.collective_compute(
                "AllGather", mybir.AluOpType.bypass, replica_groups=rg,
                ins=[ag_in[:].opt()], outs=[cur0[:].opt()])

            # ================ K propagation hops ================
            KPH = os.environ.get("DAGNN_KPH", "full")
            curbufs = [cur0, cur1]
            with (
                tc.tile_pool(name="gat", bufs=3) as gpool,
                tc.tile_pool(name="spool", bufs=4) as sbpool,
                tc.tile_pool(name="hps", bufs=4, space="PSUM") as hpps,
            ):
                for i in range(1, (0 if KPH == "noprop" else K) + 1):
                    rd = curbufs[(i - 1) % 2]
                    wr = curbufs[i % 2]
                    rcol = wR[:, K + i: K + i + 1]        # w_i / w_{i-1}
                    for g in range(NG):
                        gt = gpool.tile([128, GG * CH * 128], F32, tag="gt")
                        asl = GG * cfg.ASLOTS
                        nc.gpsimd.dma_gather(
                            gt[:, 0:asl].rearrange("p (c f) -> p c f", f=128),
                            rd[cfg.WINA_LO:cfg.WINA_LO + cfg.WINW, :],
                            idx_sb[:, g * GG * acols:(g + 1) * GG * acols],
                            asl, asl, 128, single_packet=False)
                        nc.gpsimd.dma_gather(
                            gt[:, asl:2 * asl].rearrange("p (c f) -> p c f", f=128),
                            rd[cfg.WINB_LO:cfg.WINB_LO + cfg.WINW, :],
                            idx_sb[:, (BPC + g * GG) * acols:
                                   (BPC + (g + 1) * GG) * acols],
                            asl, asl, 128, single_packet=False)
                        for j in range(GG):
                            b = g * GG + j
                            sA = sbpool.tile([128, ACH * 128], F32, tag="sA")
                            nc.vector.tensor_tensor(
                                out=sA[:].rearrange("p (c d) -> p c d", d=128),
                                in0=iota8f[:].rearrange("p (c d) -> p c d", d=128),
                                in1=dstT[:, b * ACH:(b + 1) * ACH]
                                    .rearrange("p (c d) -> p c d", d=1)
                                    .to_broadcast([128, ACH, 128]),
                                op=OP.is_equal)
                            sB = sbpool.tile([128, BCH * 128], F32, tag="sB")
                            boff = BPC * ACH
                            nc.vector.tensor_tensor(
                                out=sB[:].rearrange("p (c d) -> p c d", d=128),
                                in0=iota8f[:].rearrange("p (c d) -> p c d", d=128),
                                in1=dstT[:, boff + b * BCH: boff + (b + 1) * BCH]
                                    .rearrange("p (c d) -> p c d", d=1)
                                    .to_broadcast([128, BCH, 128]),
                                op=OP.is_equal)
                            ps = hpps.tile([128, 128], F32, space="PSUM",
                                           tag="hps")
                            for k in range(ACH):
                                nc.tensor.matmul(
                                    out=ps[:],
                                    lhsT=sA[:, k * 128:(k + 1) * 128],
                                    rhs=gt[:, (j * ACH + k) * 128:
                                           (j * ACH + k + 1) * 128],
                                    start=(k == 0), stop=False)
                            for k in range(BCH):
                                nc.tensor.matmul(
                                    out=ps[:],
                                    lhsT=sB[:, k * 128:(k + 1) * 128],
                                    rhs=gt[:, asl + (j * BCH + k) * 128:
                                           asl + (j * BCH + k + 1) * 128],
                                    start=False, stop=(k == BCH - 1))
                            tcols = slice(b * BLK, (b + 1) * BLK)
                            nc.scalar.activation(stage[:, tcols], ps[:],
                                                 AF.Copy, bias=0.0, scale=rcol)
                            nc.vector.tensor_tensor(out=accum[:, tcols],
                                                    in0=accum[:, tcols],
                                                    in1=stage[:, tcols],
                                                    op=OP.add)
                    if i < K:
                        nc.sync.dma_start(
                            ag_in[:].rearrange("(b p) f -> p b f", p=BLK),
                            stage[:].rearrange("p (b f) -> p b f", f=H))
                        if KPH == "noag":
                            nc.sync.dma_start(wr[0:ROWS, :], ag_in[:])
                        else:
                            nc.gpsimd.collective_compute(
                                "AllGather", mybir.AluOpType.bypass,
                                replica_groups=rg,
                                ins=[ag_in[:].opt()], outs=[wr[:].opt()])

            # ================ output MLP ================
            with (
                tc.tile_pool(name="tail_a", bufs=1) as tapool,
                tc.tile_pool(name="tail_t", bufs=4) as ttpool,
                tc.tile_pool(name="tail_ps", bufs=2, space="PSUM") as tpps,
                tc.tile_pool(name="tstat", bufs=1) as tspool,
            ):
                a3 = tapool.tile([64, ROWS], F32, tag="a3")
                scol3 = tspool.tile([64, BPC], F32, tag="scol3")
                qcol3 = tspool.tile([64, BPC], F32, tag="qcol3")
                bn3_sb = tspool.tile([128, 2], F32, tag="bn3sb")
                bn3st = tspool.tile([64, 6], F32, tag="bn3st")

                for t in range(BPC):
                    tcols = slice(t * BLK, (t + 1) * BLK)
                    otp = tpps.tile([128, 128], F32, space="PSUM", tag="otp")
                    nc.tensor.transpose(otp[:], accum[:, tcols], identity[:])
                    ot = ttpool.tile([128, 128], F32, tag="ot")
                    nc.scalar.copy(ot[:], otp[:])
                    zp = tpps.tile([64, 128], F32, space="PSUM", tag="zp3")
                    nc.tensor.matmul(out=zp[:], lhsT=w3sb[:], rhs=ot[:],
                                     start=True, stop=True)
                    nc.scalar.copy(a3[:, tcols], zp[:])
                    nc.vector.tensor_reduce(scol3[:, t:t + 1], a3[:, tcols],
                                            axis=mybir.AxisListType.X, op=OP.add)
                    sq = ttpool.tile([64, 128], F32, tag="sq3t")
                    nc.scalar.square(sq[:], a3[:, tcols])
                    nc.vector.tensor_reduce(qcol3[:, t:t + 1], sq[:],
                                            axis=mybir.AxisListType.X, op=OP.add)
                nc.vector.tensor_reduce(bn3_sb[:64, 0:1], scol3[:],
                                        axis=mybir.AxisListType.X, op=OP.add)
                nc.vector.tensor_reduce(bn3_sb[:64, 1:2], qcol3[:],
                                        axis=mybir.AxisListType.X, op=OP.add)
                nc.sync.dma_start(bn_in_d[2][:64, :], bn3_sb[:64, :])
                zf = tspool.tile([64, 2], F32, tag="zf3")
                nc.vector.memset(zf[:], 0.0)
                nc.sync.dma_start(bn_in_d[2][64:, :], zf[:])
                nc.gpsimd.collective_compute(
                    "AllReduce", OP.add, replica_groups=rg,
                    ins=[bn_in_d[2][:].opt()], outs=[bn_out_d[2][:].opt()])
                nc.sync.dma_start(bn3_sb[:64, :], bn_out_d[2][:64, :])
                nc.vector.tensor_tensor(out=bn3_sb[:64, :], in0=bn3_sb[:64, :],
                                        in1=zpad3_keep[:], op=OP.add)
                st = bn3st
                nc.scalar.mul(st[:, 0:1], bn3_sb[:64, 0:1], invN)
                nc.scalar.mul(st[:, 1:2], bn3_sb[:64, 1:2], invN)
                nc.vector.tensor_tensor(out=st[:, 2:3], in0=st[:, 0:1],
                                        in1=st[:, 0:1], op=OP.mult)
                nc.vector.tensor_tensor(out=st[:, 2:3], in0=st[:, 1:2],
                                        in1=st[:, 2:3], op=OP.subtract)
                nc.scalar.activation(st[:, 3:4], st[:, 2:3], AF.Sqrt,
                                     bias=epsc[:64, 0:1], scale=1.0)
                nc.vector.reciprocal(st[:, 4:5], st[:, 3:4])
                nc.vector.tensor_tensor(out=st[:, 4:5], in0=st[:, 4:5],
                                        in1=g3sb[:, 0:1], op=OP.mult)
                nc.vector.tensor_tensor(out=st[:, 5:6], in0=st[:, 0:1],
                                        in1=st[:, 4:5], op=OP.mult)
                nc.vector.tensor_tensor(out=st[:, 5:6], in0=be3sb[:, 0:1],
                                        in1=st[:, 5:6], op=OP.subtract)

                for t in range(BPC):
                    tcols = slice(t * BLK, (t + 1) * BLK)
                    nc.scalar.activation(a3[:, tcols], a3[:, tcols], AF.Relu,
                                         bias=st[:, 5:6], scale=st[:, 4:5])
                    po_a = tpps.tile([128, OUT_A], F32, space="PSUM", tag="poa")
                    nc.tensor.matmul(out=po_a[:], lhsT=a3[:, tcols],
                                     rhs=wosb[:, 0:OUT_A], start=True, stop=True)
                    ost = ttpool.tile([128, OUT], F32, tag="ost")
                    nc.vector.tensor_tensor(out=ost[:, 0:OUT_A], in0=po_a[:],
                                            in1=boutR[:, 0:OUT_A], op=OP.add)
                    if OUT_B:
                        po_b = tpps.tile([128, OUT_B], F32, space="PSUM",
                                         tag="pob")
                        nc.tensor.matmul(out=po_b[:], lhsT=a3[:, tcols],
                                         rhs=wosb[:, OUT_A:OUT], start=True,
                                         stop=True)
                        nc.vector.tensor_tensor(out=ost[:, OUT_A:OUT],
                                                in0=po_b[:],
                                                in1=boutR[:, OUT_A:OUT],
                                                op=OP.add)
                    nc.sync.dma_start(out[t * BLK:(t + 1) * BLK, :], ost[:])

    nc.compile()
    return nc


# ---------------------------------------------------------------- runner

_CACHE = {}


def run(inputs: dict, cfg: Cfg, trace: bool = False):
    from concourse.bass_utils import run_bass_kernel_spmd

    edge_index = np.asarray(inputs["edge_index"])
    perm, idx_img, dst_img = build_edge_structures(cfg, edge_index)

    x = np.asarray(inputs["x"], np.float32)
    xp = np.zeros((cfg.NP, cfg.IN), np.float32)
    xp[perm[:cfg.N]] = x

    def col(v, parts):
        return np.asarray(v, np.float32).reshape(parts, 1)

    in_maps = []
    for c in range(cfg.NCORES):
        in_maps.append({
            "x_sh": xp[c * cfg.ROWS:(c + 1) * cfg.ROWS],
            "idxs": idx_img[c],
            "drel": dst_img[c],
            "W1": np.asarray(inputs["W1"], np.float32),
            "W2": np.asarray(inputs["W2"], np.float32),
            "W3": np.asarray(inputs["W3"], np.float32),
            "Wout": np.asarray(inputs["Wout"], np.float32),
            "g1": col(inputs["g1"], 128), "be1": col(inputs["be1"], 128),
            "g2": col(inputs["g2"], 128), "be2": col(inputs["be2"], 128),
            "g3": col(inputs["g3"], 64), "be3": col(inputs["be3"], 64),
            "att": np.asarray(inputs["att"], np.float32).reshape(1, -1),
            "bout": np.asarray(inputs["bout"], np.float32).reshape(1, -1),
        })

    key = (cfg.N, cfg.E, cfg.K, cfg.BPC, cfg.OUT)
    if key not in _CACHE:
        _CACHE[key] = build_nc(cfg)
    nc = _CACHE[key]

    res = run_bass_kernel_spmd(nc, in_maps, core_ids=list(range(cfg.NCORES)),
                               trace=trace)
    outp = np.concatenate([res.results[c]["out"] for c in range(cfg.NCORES)], 0)
    outf = outp[perm[:cfg.N]]
    return outf.astype(np.float32), res


def kernel(**inputs) -> np.ndarray:
    out, _ = run(inputs, FULL)
    return out



# revision 20
# speedup vs baseline: 1.8629x; 1.8629x over previous
"""DAGNN recommender forward pass on 8 Trainium2 NeuronCores (Bass/Tile).

Strategy (nodes sharded across cores, dst-block scatter via selection-matrix
matmuls, per-hop AllGather of the updated node states):

 - Nodes are relabeled by a host-side permutation so each 128-row "block"
   has a balanced in-edge count (<= 2048).  400 blocks total, 50 per core.
 - Per hop, each core gathers cur[src] rows for its ~100k in-edges with
   `dma_gather` (512B rows from a replicated DRAM copy of cur), then for
   every dst block accumulates 16 matmuls  psum += S_chunk.T @ G_chunk
   where S is the one-hot (edge -> dst row) selection matrix built on the
   fly with iota + is_equal.  Gather indices are int16, so gathers read
   from two 32768-row windows of cur (A = rows [0, 32768), B = last 32768
   rows); the host assigns each edge to a window.
 - softmax(att) hop weights are folded into cur (cur'_i = w_i A^i h), so
   the DAGNN output is just the running sum of cur' slices.
 - The input/output MLPs run feature-major (activations transposed) so
   BatchNorm's per-feature statistics live on partitions; train-mode BN
   makes the b1/b2/b3 biases cancel, and the zero-padded fake nodes are
   corrected analytically in the BN statistics.
 - Cross-core traffic: one AllGather of the cur slices per hop plus three
   tiny AllReduces for BN statistics.
"""

import os
import sys

if "/opt/trn_rl_repo" not in sys.path:
    sys.path.insert(0, "/opt/trn_rl_repo")

import numpy as np


# ---------------------------------------------------------------- config

class Cfg:
    def __init__(self, N=50000, E=800000, K=10, BPC=50, OUT=1000):
        self.N, self.E, self.K, self.BPC, self.OUT = N, E, K, BPC, OUT
        self.H = 128
        self.IN = 128
        self.NCORES = 8
        self.BLK = 128
        self.NBLOCKS = self.NCORES * BPC
        self.NP = self.NBLOCKS * self.BLK
        self.ROWS = BPC * self.BLK              # rows per core
        self.ACH = 8
        self.BCH = 8
        self.CH = self.ACH + self.BCH
        self.ASLOTS = self.ACH * self.BLK       # 1024
        self.BSLOTS = self.BCH * self.BLK
        self.WINW = min(32768, self.NP)
        self.WINA_LO = 0
        self.WINB_LO = self.NP - self.WINW
        self.NPADN = self.NP - N
        self.GG = 2                              # blocks per gather group
        assert BPC % self.GG == 0
        self.NGROUPS = BPC // self.GG
        self.EPS = 1e-5


FULL = Cfg()


# ---------------------------------------------------------------- host prep

def balance_nodes(cfg: Cfg, dst: np.ndarray) -> np.ndarray:
    """perm: old node id (incl. pads) -> new padded row id, balancing
    per-block in-edge counts (LPT greedy)."""
    import heapq
    deg = np.bincount(dst, minlength=cfg.N).astype(np.int64)
    deg_all = np.concatenate([deg, np.zeros(cfg.NP - cfg.N, np.int64)])
    order = np.argsort(-deg_all, kind="stable")
    load = np.zeros(cfg.NBLOCKS, np.int64)
    fill = np.zeros(cfg.NBLOCKS, np.int64)
    perm = np.empty(cfg.NP, np.int64)
    heap = [(0, b) for b in range(cfg.NBLOCKS)]
    heapq.heapify(heap)
    for node in order:
        while True:
            _, b = heapq.heappop(heap)
            if fill[b] < cfg.BLK:
                break
        perm[node] = b * cfg.BLK + fill[b]
        fill[b] += 1
        load[b] += deg_all[node]
        if fill[b] < cfg.BLK:
            heapq.heappush(heap, (load[b], b))
    return perm


def build_edge_structures(cfg: Cfg, edge_index: np.ndarray):
    """(perm, idx_img [8,128,BPC*128] int16, dst_img [8,128,BPC*16] f32)."""
    src, dst = edge_index[0].astype(np.int64), edge_index[1].astype(np.int64)
    perm = balance_nodes(cfg, dst)
    psrc = perm[src]
    pdst = perm[dst]
    blk = pdst // cfg.BLK
    rel = pdst % cfg.BLK
    order = np.argsort(blk, kind="stable")
    psrc, rel, blk = psrc[order], rel[order], blk[order]
    starts = np.searchsorted(blk, np.arange(cfg.NBLOCKS + 1))

    acols = cfg.ASLOTS // 16                    # idx cols per block per region
    idx_img = np.zeros((cfg.NCORES, 16, 2 * cfg.BPC * acols), np.int16)
    dst_img = np.full((cfg.NCORES, 128, cfg.BPC * cfg.CH), -1.0, np.float32)
    aoff = cfg.BPC * acols                      # col offset of B region

    for b in range(cfg.NBLOCKS):
        s, e = starts[b], starts[b + 1]
        bs, br = psrc[s:e], rel[s:e]
        assert e - s <= cfg.ASLOTS + cfg.BSLOTS, (b, e - s)
        mustA = bs < cfg.WINB_LO
        mustB = bs >= cfg.WINW
        flex = ~mustA & ~mustB
        nA, nB, nF = int(mustA.sum()), int(mustB.sum()), int(flex.sum())
        assert nA <= cfg.ASLOTS and nB <= cfg.BSLOTS, (b, nA, nB)
        takeA = min(cfg.ASLOTS - nA, nF)
        assert nB + (nF - takeA) <= cfg.BSLOTS, (b, nA, nB, nF)
        fi = np.nonzero(flex)[0]
        a_sel = np.concatenate([np.nonzero(mustA)[0], fi[:takeA]])
        b_sel = np.concatenate([np.nonzero(mustB)[0], fi[takeA:]])

        a_idx = np.zeros(cfg.ASLOTS, np.int64)
        a_rel = np.full(cfg.ASLOTS, -1.0, np.float32)
        a_idx[:len(a_sel)] = bs[a_sel] - cfg.WINA_LO
        a_rel[:len(a_sel)] = br[a_sel]
        b_idx = np.zeros(cfg.BSLOTS, np.int64)
        b_rel = np.full(cfg.BSLOTS, -1.0, np.float32)
        b_idx[:len(b_sel)] = bs[b_sel] - cfg.WINB_LO
        b_rel[:len(b_sel)] = br[b_sel]
        assert 0 <= a_idx.min(initial=0) and a_idx.max(initial=0) < 32768
        assert 0 <= b_idx.min(initial=0) and b_idx.max(initial=0) < 32768

        c, lb = divmod(b, cfg.BPC)
        idx_img[c, :, lb * acols:(lb + 1) * acols] = \
            a_idx.reshape(acols, 16).T.astype(np.int16)
        idx_img[c, :, aoff + lb * acols: aoff + (lb + 1) * acols] = \
            b_idx.reshape(acols, 16).T.astype(np.int16)
        dst_img[c, :, lb * cfg.ACH:(lb + 1) * cfg.ACH] = \
            a_rel.reshape(cfg.ACH, 128).T
        boff = cfg.BPC * cfg.ACH
        dst_img[c, :, boff + lb * cfg.BCH: boff + (lb + 1) * cfg.BCH] = \
            b_rel.reshape(cfg.BCH, 128).T

    idx_img = np.tile(idx_img, (1, 8, 1))       # replicate to 128 partitions
    # 16x-expanded dst image (each rel-dst value repeated 16x along cols) so
    # the on-chip is_equal runs with a packed last dim (DVE 2x_1p mode).
    dst_exp = np.repeat(dst_img, 16, axis=2)
    return perm, idx_img, dst_exp


# ---------------------------------------------------------------- device

def build_nc(cfg: Cfg):
    from concourse import bass, mybir, bacc, tile

    F32 = mybir.dt.float32
    BF16 = mybir.dt.bfloat16
    I16 = mybir.dt.int16
    I32 = mybir.dt.int32
    AF = mybir.ActivationFunctionType
    OP = mybir.AluOpType
    BLK, BPC, CH, ACH, BCH = cfg.BLK, cfg.BPC, cfg.CH, cfg.ACH, cfg.BCH
    ROWS, NP, K, H, OUT = cfg.ROWS, cfg.NP, cfg.K, cfg.H, cfg.OUT
    GG, NG = cfg.GG, cfg.NGROUPS
    acols = cfg.ASLOTS // 16
    NCORES = cfg.NCORES
    invN = 1.0 / cfg.N
    OUT_A = min(512, OUT)
    OUT_B = OUT - OUT_A

    nc = bacc.Bacc("TRN2", target_bir_lowering=False, debug=False,
                   num_devices=NCORES, num_swdge_queues=4)

    x_sh = nc.dram_tensor("x_sh", [ROWS, cfg.IN], F32, kind="ExternalInput")
    idxs = nc.dram_tensor("idxs", [128, 2 * BPC * acols], I16, kind="ExternalInput")
    drel = nc.dram_tensor("drel", [128, BPC * CH * 16], BF16, kind="ExternalInput")
    W1 = nc.dram_tensor("W1", [cfg.IN, H], F32, kind="ExternalInput")
    W2 = nc.dram_tensor("W2", [H, H], F32, kind="ExternalInput")
    W3 = nc.dram_tensor("W3", [H, H // 2], F32, kind="ExternalInput")
    Wout = nc.dram_tensor("Wout", [H // 2, OUT], BF16, kind="ExternalInput")
    g1 = nc.dram_tensor("g1", [H, 1], F32, kind="ExternalInput")
    be1 = nc.dram_tensor("be1", [H, 1], F32, kind="ExternalInput")
    g2 = nc.dram_tensor("g2", [H, 1], F32, kind="ExternalInput")
    be2 = nc.dram_tensor("be2", [H, 1], F32, kind="ExternalInput")
    g3 = nc.dram_tensor("g3", [H // 2, 1], F32, kind="ExternalInput")
    be3 = nc.dram_tensor("be3", [H // 2, 1], F32, kind="ExternalInput")
    att = nc.dram_tensor("att", [1, K + 1], F32, kind="ExternalInput")
    bout = nc.dram_tensor("bout", [1, OUT], F32, kind="ExternalInput")
    out = nc.dram_tensor("out", [ROWS, OUT], F32, kind="ExternalOutput")

    rg = [list(range(NCORES))]

    with tile.TileContext(nc) as tc:
        with (
            tc.tile_pool(name="const", bufs=1) as cpool,
            tc.tile_pool(name="resid", bufs=1) as rpool,
            tc.tile_pool(name="dram", bufs=1, space="DRAM") as dpool,
        ):
            # ---------------- persistent DRAM buffers
            cur0 = dpool.tile([NP, H], BF16, tag="cur0", addr_space="Shared")
            cur1 = dpool.tile([NP, H], BF16, tag="cur1", addr_space="Shared")
            ag_in = dpool.tile([ROWS, H], BF16, tag="ag_in")
            bn_in_d = [dpool.tile([H, 2], F32, tag=f"bni{i}", name=f"bni{i}") for i in range(3)]
            bn_out_d = [dpool.tile([H, 2], F32, tag=f"bno{i}", name=f"bno{i}") for i in range(3)]

            # ---------------- constants / weights to SBUF
            w1sb = cpool.tile([cfg.IN, H], F32)
            nc.sync.dma_start(w1sb[:], W1[:])
            w2sb = cpool.tile([H, H], F32)
            nc.sync.dma_start(w2sb[:], W2[:])
            w3sb = cpool.tile([H, H // 2], F32)
            nc.sync.dma_start(w3sb[:], W3[:])
            wosb = cpool.tile([H // 2, OUT], BF16)
            nc.sync.dma_start(wosb[:], Wout[:])
            g1sb = cpool.tile([H, 1], F32); nc.sync.dma_start(g1sb[:], g1[:])
            be1sb = cpool.tile([H, 1], F32); nc.sync.dma_start(be1sb[:], be1[:])
            g2sb = cpool.tile([H, 1], F32); nc.sync.dma_start(g2sb[:], g2[:])
            be2sb = cpool.tile([H, 1], F32); nc.sync.dma_start(be2sb[:], be2[:])
            g3sb = cpool.tile([H // 2, 1], F32); nc.sync.dma_start(g3sb[:], g3[:])
            be3sb = cpool.tile([H // 2, 1], F32); nc.sync.dma_start(be3sb[:], be3[:])
            attsb = cpool.tile([1, K + 1], F32); nc.sync.dma_start(attsb[:], att[:])
            bosb = cpool.tile([1, OUT], F32); nc.sync.dma_start(bosb[:], bout[:])

            idx_sb = cpool.tile([128, 2 * BPC * acols], I16)
            nc.sync.dma_start(idx_sb[:], idxs[:])
            dstT = cpool.tile([128, BPC * CH * 16], BF16)
            nc.sync.dma_start(dstT[:], drel[:])

            identity = cpool.tile([128, 128], F32)
            from concourse.masks import make_identity
            make_identity(nc, identity[:])

            iota_i = cpool.tile([128, ACH * 128], I32)
            nc.gpsimd.iota(iota_i[:].rearrange("p (c d) -> p c d", d=128),
                           pattern=[[0, ACH], [1, 128]], base=0,
                           channel_multiplier=0)
            iota8f = cpool.tile([128, ACH * 128], F32)
            nc.vector.tensor_copy(iota8f[:], iota_i[:])
            iota_bf = cpool.tile([128, ACH * 128], BF16)
            nc.vector.tensor_copy(iota_bf[:], iota8f[:])

            ones1 = cpool.tile([1, 128], F32)
            nc.vector.memset(ones1[:], 1.0)
            epsc = cpool.tile([128, 1], F32)
            nc.vector.memset(epsc[:], cfg.EPS)

            # softmax(att) -> w[0..K], ratios r_i = w_i/w_{i-1} -> wR [128, 2K+1]
            mx = cpool.tile([1, 1], F32)
            nc.vector.tensor_reduce(mx[:], attsb[:], axis=mybir.AxisListType.X,
                                    op=OP.max)
            nmx = cpool.tile([1, 1], F32)
            nc.scalar.mul(nmx[:], mx[:], -1.0)
            ew = cpool.tile([1, K + 1], F32)
            nc.scalar.activation(ew[:], attsb[:], AF.Exp, bias=nmx[:, 0:1],
                                 scale=1.0)
            ssum = cpool.tile([1, 1], F32)
            nc.vector.tensor_reduce(ssum[:], ew[:], axis=mybir.AxisListType.X,
                                    op=OP.add)
            rsum = cpool.tile([1, 1], F32)
            nc.vector.reciprocal(rsum[:], ssum[:])
            wv = cpool.tile([1, K + 1], F32)
            nc.vector.tensor_scalar_mul(wv[:], ew[:], rsum[:, 0:1])
            rw = cpool.tile([1, K + 1], F32)
            nc.vector.reciprocal(rw[:], wv[:])
            wcat = cpool.tile([1, 2 * K + 1], F32)
            nc.vector.tensor_copy(wcat[:, 0:K + 1], wv[:])
            nc.vector.tensor_tensor(out=wcat[:, K + 1:2 * K + 1],
                                    in0=wv[:, 1:K + 1], in1=rw[:, 0:K],
                                    op=OP.mult)
            with tc.tile_pool(name="wps", bufs=1, space="PSUM") as wps:
                wpsum = wps.tile([128, 2 * K + 1], F32, space="PSUM")
                nc.tensor.matmul(out=wpsum[:], lhsT=ones1[:], rhs=wcat[:],
                                 start=True, stop=True)
                wR = cpool.tile([128, 2 * K + 1], F32)
                nc.scalar.copy(wR[:], wpsum[:])

                # bout replicated to 128 partitions
                boutR = cpool.tile([128, OUT], F32)
                bps_a = wps.tile([128, OUT_A], F32, space="PSUM", tag="bps")
                nc.tensor.matmul(out=bps_a[:], lhsT=ones1[:],
                                 rhs=bosb[:, 0:OUT_A], start=True, stop=True)
                nc.scalar.copy(boutR[:, 0:OUT_A], bps_a[:])
                if OUT_B:
                    bps_b = wps.tile([128, OUT_B], F32, space="PSUM", tag="bps")
                    nc.tensor.matmul(out=bps_b[:], lhsT=ones1[:],
                                     rhs=bosb[:, OUT_A:OUT], start=True,
                                     stop=True)
                    nc.scalar.copy(boutR[:, OUT_A:OUT], bps_b[:])

            # ---------------- resident activations
            stage = rpool.tile([128, ROWS], BF16, tag="stage")   # node-major cur'
            accum = rpool.tile([128, ROWS], F32, tag="accum")    # sum of cur'

            # ================ input MLP (feature-major) ================
            with (
                tc.tile_pool(name="mlp_a", bufs=1) as apool,
                tc.tile_pool(name="mlp_t", bufs=4) as tpool,
                tc.tile_pool(name="mlp_ps", bufs=3, space="PSUM") as mpps,
                tc.tile_pool(name="stat", bufs=1) as spool,
            ):
                a1 = apool.tile([128, ROWS], F32, tag="a1")      # z1T then h1T
                a2 = apool.tile([128, ROWS], F32, tag="a2")      # z2T then h2T
                scol = spool.tile([128, BPC], F32, tag="scol")
                qcol = spool.tile([128, BPC], F32, tag="qcol")
                bn_sb = [spool.tile([128, 2], F32, tag=f"bnsb{i}", name=f"bnsb{i}")
                         for i in range(3)]
                bnst = [spool.tile([128, 6], F32, tag=f"bnst{i}", name=f"bnst{i}")
                        for i in range(3)]  # mean, ex2, var, std, scale, shift

                def bn_stats_finish(i, gsb, besb, parts=128):
                    """bn_sb[i][:,0]=sum, [:,1]=sumsq (already AllReduced &
                    pad-corrected) -> bnst[i] cols: scale(4), shift(5)."""
                    st = bnst[i]
                    nc.scalar.mul(st[:parts, 0:1], bn_sb[i][:parts, 0:1], invN)
                    nc.scalar.mul(st[:parts, 1:2], bn_sb[i][:parts, 1:2], invN)
                    nc.vector.tensor_tensor(out=st[:parts, 2:3],
                                            in0=st[:parts, 0:1],
                                            in1=st[:parts, 0:1], op=OP.mult)
                    nc.vector.tensor_tensor(out=st[:parts, 2:3],
                                            in0=st[:parts, 1:2],
                                            in1=st[:parts, 2:3], op=OP.subtract)
                    nc.scalar.activation(st[:parts, 3:4], st[:parts, 2:3],
                                         AF.Sqrt, bias=epsc[:parts, 0:1],
                                         scale=1.0)
                    nc.vector.reciprocal(st[:parts, 4:5], st[:parts, 3:4])
                    nc.vector.tensor_tensor(out=st[:parts, 4:5],
                                            in0=st[:parts, 4:5],
                                            in1=gsb[:parts, 0:1], op=OP.mult)
                    nc.vector.tensor_tensor(out=st[:parts, 5:6],
                                            in0=st[:parts, 0:1],
                                            in1=st[:parts, 4:5], op=OP.mult)
                    nc.vector.tensor_tensor(out=st[:parts, 5:6],
                                            in0=besb[:parts, 0:1],
                                            in1=st[:parts, 5:6], op=OP.subtract)

                def bn_allreduce(i, parts=128):
                    nc.sync.dma_start(bn_in_d[i][:parts, :], bn_sb[i][:parts, :])
                    if parts < 128:
                        zf = spool.tile([128 - parts, 2], F32, tag="zfill")
                        nc.vector.memset(zf[:], 0.0)
                        nc.sync.dma_start(bn_in_d[i][parts:, :], zf[:])
                    nc.gpsimd.collective_compute(
                        "AllReduce", OP.add, replica_groups=rg,
                        ins=[bn_in_d[i][:].opt()], outs=[bn_out_d[i][:].opt()])
                    nc.sync.dma_start(bn_sb[i][:parts, :], bn_out_d[i][:parts, :])

                # ---- MLP1: z1T = W1.T @ xT
                for t in range(BPC):
                    xe = tpool.tile([128, 128], F32, tag="xload")
                    nc.sync.dma_start(xe[:], x_sh[t * BLK:(t + 1) * BLK, :])
                    xtp = mpps.tile([128, 128], F32, space="PSUM", tag="xtp")
                    nc.tensor.transpose(xtp[:], xe[:], identity[:])
                    xt = tpool.tile([128, 128], F32, tag="xt")
                    nc.scalar.copy(xt[:], xtp[:])
                    zp = mpps.tile([128, 128], F32, space="PSUM", tag="zp")
                    nc.tensor.matmul(out=zp[:], lhsT=w1sb[:], rhs=xt[:],
                                     start=True, stop=True)
                    tcols = slice(t * BLK, (t + 1) * BLK)
                    nc.scalar.copy(a1[:, tcols], zp[:])
                    nc.vector.tensor_reduce(scol[:, t:t + 1], a1[:, tcols],
                                            axis=mybir.AxisListType.X, op=OP.add)
                    sq = tpool.tile([128, 128], F32, tag="sq")
                    nc.scalar.square(sq[:], a1[:, tcols])
                    nc.vector.tensor_reduce(qcol[:, t:t + 1], sq[:],
                                            axis=mybir.AxisListType.X, op=OP.add)
                nc.vector.tensor_reduce(bn_sb[0][:, 0:1], scol[:],
                                        axis=mybir.AxisListType.X, op=OP.add)
                nc.vector.tensor_reduce(bn_sb[0][:, 1:2], qcol[:],
                                        axis=mybir.AxisListType.X, op=OP.add)
                bn_allreduce(0)
                bn_stats_finish(0, g1sb, be1sb)
                sc1 = bnst[0][:, 4:5]
                sh1 = bnst[0][:, 5:6]
                # h1T = relu(sc1 * z1T + sh1), in place
                for t in range(BPC):
                    tcols = slice(t * BLK, (t + 1) * BLK)
                    nc.scalar.activation(a1[:, tcols], a1[:, tcols], AF.Relu,
                                         bias=sh1, scale=sc1)

                # pad row value hpad1 = relu(sh1)
                hpad1 = spool.tile([128, 1], F32, tag="hpad1")
                nc.scalar.activation(hpad1[:], sh1, AF.Relu)
                zpad2 = spool.tile([128, 1], F32, tag="zpad2")
                zp2p = mpps.tile([128, 1], F32, space="PSUM", tag="zp")
                nc.tensor.matmul(out=zp2p[:], lhsT=w2sb[:], rhs=hpad1[:],
                                 start=True, stop=True)
                nc.scalar.copy(zpad2[:], zp2p[:])

                # ---- MLP2: z2T = W2.T @ h1T
                for t in range(BPC):
                    tcols = slice(t * BLK, (t + 1) * BLK)
                    zp = mpps.tile([128, 128], F32, space="PSUM", tag="zp")
                    nc.tensor.matmul(out=zp[:], lhsT=w2sb[:], rhs=a1[:, tcols],
                                     start=True, stop=True)
                    nc.scalar.copy(a2[:, tcols], zp[:])
                    nc.vector.tensor_reduce(scol[:, t:t + 1], a2[:, tcols],
                                            axis=mybir.AxisListType.X, op=OP.add)
                    sq = tpool.tile([128, 128], F32, tag="sq")
                    nc.scalar.square(sq[:], a2[:, tcols])
                    nc.vector.tensor_reduce(qcol[:, t:t + 1], sq[:],
                                            axis=mybir.AxisListType.X, op=OP.add)
                nc.vector.tensor_reduce(bn_sb[1][:, 0:1], scol[:],
                                        axis=mybir.AxisListType.X, op=OP.add)
                nc.vector.tensor_reduce(bn_sb[1][:, 1:2], qcol[:],
                                        axis=mybir.AxisListType.X, op=OP.add)
                # pad correction (only core 0 contributes it, pre-AllReduce)
                # every core computes sum over its own rows; pads exist on all
                # cores' slices, already included -> subtract npad * zpad2
                # AFTER AllReduce.  Simpler: subtract (npad/NCORES-weighted)?
                # pads per core vary; correct GLOBALLY after the AllReduce.
                bn_allreduce(1)
                corr = spool.tile([128, 2], F32, tag="corr")
                nc.scalar.mul(corr[:, 0:1], zpad2[:], -float(cfg.NPADN))
                sqz = spool.tile([128, 1], F32, tag="sqz")
                nc.scalar.square(sqz[:], zpad2[:])
                nc.scalar.mul(corr[:, 1:2], sqz[:], -float(cfg.NPADN))
                nc.vector.tensor_tensor(out=bn_sb[1][:], in0=bn_sb[1][:],
                                        in1=corr[:], op=OP.add)
                bn_stats_finish(1, g2sb, be2sb)
                sc2 = bnst[1][:, 4:5]
                sh2 = bnst[1][:, 5:6]
                # h2T = relu(sc2*z2T + sh2) + h1T   (a2 in place)
                for t in range(BPC):
                    tcols = slice(t * BLK, (t + 1) * BLK)
                    nc.scalar.activation(a2[:, tcols], a2[:, tcols], AF.Relu,
                                         bias=sh2, scale=sc2)
                    nc.vector.tensor_tensor(out=a2[:, tcols], in0=a2[:, tcols],
                                            in1=a1[:, tcols], op=OP.add)

                # hpad2 = relu(sc2*zpad2+sh2) + hpad1 ; zpad3 = (w0*hpad2)@W3
                hpad2 = spool.tile([128, 1], F32, tag="hpad2")
                nc.scalar.activation(hpad2[:], zpad2[:], AF.Relu, bias=sh2,
                                     scale=sc2)
                nc.vector.tensor_tensor(out=hpad2[:], in0=hpad2[:],
                                        in1=hpad1[:], op=OP.add)
                w0hpad2 = spool.tile([128, 1], F32, tag="w0hpad2")
                nc.scalar.activation(w0hpad2[:], hpad2[:], AF.Copy, bias=0.0,
                                     scale=wR[:, 0:1])
                zpad3 = spool.tile([64, 1], F32, tag="zpad3")
                zp3p = mpps.tile([64, 1], F32, space="PSUM", tag="zp")
                nc.tensor.matmul(out=zp3p[:], lhsT=w3sb[:], rhs=w0hpad2[:],
                                 start=True, stop=True)
                nc.scalar.copy(zpad3[:], zp3p[:])
                zpad3_keep = rpool.tile([64, 2], F32, tag="zpad3k")
                nc.scalar.mul(zpad3_keep[:, 0:1], zpad3[:], -float(cfg.NPADN))
                sq3 = spool.tile([64, 1], F32, tag="sq3")
                nc.scalar.square(sq3[:], zpad3[:])
                nc.scalar.mul(zpad3_keep[:, 1:2], sq3[:], -float(cfg.NPADN))

                # stage = h2 node-major; accum = w0 * stage
                for t in range(BPC):
                    tcols = slice(t * BLK, (t + 1) * BLK)
                    htp = mpps.tile([128, 128], F32, space="PSUM", tag="xtp")
                    nc.tensor.transpose(htp[:], a2[:, tcols], identity[:])
                    nc.scalar.copy(stage[:, tcols], htp[:])
                    nc.scalar.activation(accum[:, tcols], stage[:, tcols],
                                         AF.Copy, bias=0.0, scale=wR[:, 0:1])

            # write cur'_0 and AllGather it
            nc.sync.dma_start(
                ag_in[:].rearrange("(b p) f -> p b f", p=BLK),
                stage[:].rearrange("p (b f) -> p b f", f=H))
            nc.gpsimd.collective_compute(
                "AllGather", mybir.AluOpType.bypass, replica_groups=rg,
                ins=[ag_in[:].opt()], outs=[cur0[:].opt()])

            # ================ K propagation hops ================
            KPH = os.environ.get("DAGNN_KPH", "full")
            curbufs = [cur0, cur1]
            with (
                tc.tile_pool(name="gat", bufs=4) as gpool,
                tc.tile_pool(name="spool", bufs=4) as sbpool,
                tc.tile_pool(name="hps", bufs=4, space="PSUM") as hpps,
            ):
                for i in range(1, (0 if KPH == "noprop" else K) + 1):
                    rd = curbufs[(i - 1) % 2]
                    wr = curbufs[i % 2]
                    rcol = wR[:, K + i: K + i + 1]        # w_i / w_{i-1}
                    for g in range(NG):
                        gt = gpool.tile([128, GG * CH * 128], BF16, tag="gt")
                        asl = GG * cfg.ASLOTS
                        nc.gpsimd.dma_gather(
                            gt[:, 0:asl].rearrange("p (c f) -> p c f", f=128),
                            rd[cfg.WINA_LO:cfg.WINA_LO + cfg.WINW, :],
                            idx_sb[:, g * GG * acols:(g + 1) * GG * acols],
                            asl, asl, 128, single_packet=False,
                            queue_num=(2 * g) % 4)
                        nc.gpsimd.dma_gather(
                            gt[:, asl:2 * asl].rearrange("p (c f) -> p c f", f=128),
                            rd[cfg.WINB_LO:cfg.WINB_LO + cfg.WINW, :],
                            idx_sb[:, (BPC + g * GG) * acols:
                                   (BPC + (g + 1) * GG) * acols],
                            asl, asl, 128, single_packet=False,
                            queue_num=(2 * g + 1) % 4)
                        for j in range(GG):
                            b = g * GG + j
                            sA = sbpool.tile([128, ACH * 128], BF16, tag="sA")
                            nc.vector.tensor_tensor(
                                out=sA[:].rearrange("p (c e r) -> p c e r",
                                                    e=8, r=16),
                                in0=iota_bf[:].rearrange("p (c e r) -> p c e r",
                                                         e=8, r=16),
                                in1=dstT[:, b * ACH * 16:(b + 1) * ACH * 16]
                                    .rearrange("p (c r) -> p c r", r=16)
                                    .unsqueeze(2)
                                    .to_broadcast([128, ACH, 8, 16]),
                                op=OP.is_equal)
                            sB = sbpool.tile([128, BCH * 128], BF16, tag="sB")
                            boff = BPC * ACH * 16
                            nc.vector.tensor_tensor(
                                out=sB[:].rearrange("p (c e r) -> p c e r",
                                                    e=8, r=16),
                                in0=iota_bf[:].rearrange("p (c e r) -> p c e r",
                                                         e=8, r=16),
                                in1=dstT[:, boff + b * BCH * 16:
                                         boff + (b + 1) * BCH * 16]
                                    .rearrange("p (c r) -> p c r", r=16)
                                    .unsqueeze(2)
                                    .to_broadcast([128, BCH, 8, 16]),
                                op=OP.is_equal)
                            ps = hpps.tile([128, 128], F32, space="PSUM",
                                           tag="hps")
                            for k in range(ACH):
                                nc.tensor.matmul(
                                    out=ps[:],
                                    lhsT=sA[:, k * 128:(k + 1) * 128],
                                    rhs=gt[:, (j * ACH + k) * 128:
                                           (j * ACH + k + 1) * 128],
                                    start=(k == 0), stop=False)
                            for k in range(BCH):
                                nc.tensor.matmul(
                                    out=ps[:],
                                    lhsT=sB[:, k * 128:(k + 1) * 128],
                                    rhs=gt[:, asl + (j * BCH + k) * 128:
                                           asl + (j * BCH + k + 1) * 128],
                                    start=False, stop=(k == BCH - 1))
                            tcols = slice(b * BLK, (b + 1) * BLK)
                            nc.scalar.activation(stage[:, tcols], ps[:],
                                                 AF.Copy, bias=0.0, scale=rcol)
                            nc.vector.tensor_tensor(out=accum[:, tcols],
                                                    in0=accum[:, tcols],
                                                    in1=stage[:, tcols],
                                                    op=OP.add)
                    if i < K:
                        nc.sync.dma_start(
                            ag_in[:].rearrange("(b p) f -> p b f", p=BLK),
                            stage[:].rearrange("p (b f) -> p b f", f=H))
                        if KPH == "noag":
                            nc.sync.dma_start(wr[0:ROWS, :], ag_in[:])
                        else:
                            nc.gpsimd.collective_compute(
                                "AllGather", mybir.AluOpType.bypass,
                                replica_groups=rg,
                                ins=[ag_in[:].opt()], outs=[wr[:].opt()])

            # ================ output MLP ================
            with (
                tc.tile_pool(name="tail_a", bufs=1) as tapool,
                tc.tile_pool(name="tail_t", bufs=4) as ttpool,
                tc.tile_pool(name="tail_ps", bufs=2, space="PSUM") as tpps,
                tc.tile_pool(name="tstat", bufs=1) as tspool,
            ):
                a3 = tapool.tile([64, ROWS], F32, tag="a3")
                a3h = tapool.tile([64, ROWS], BF16, tag="a3h")
                scol3 = tspool.tile([64, BPC], F32, tag="scol3")
                qcol3 = tspool.tile([64, BPC], F32, tag="qcol3")
                bn3_sb = tspool.tile([128, 2], F32, tag="bn3sb")
                bn3st = tspool.tile([64, 6], F32, tag="bn3st")

                for t in range(BPC):
                    tcols = slice(t * BLK, (t + 1) * BLK)
                    otp = tpps.tile([128, 128], F32, space="PSUM", tag="otp")
                    nc.tensor.transpose(otp[:], accum[:, tcols], identity[:])
                    ot = ttpool.tile([128, 128], F32, tag="ot")
                    nc.scalar.copy(ot[:], otp[:])
                    zp = tpps.tile([64, 128], F32, space="PSUM", tag="zp3")
                    nc.tensor.matmul(out=zp[:], lhsT=w3sb[:], rhs=ot[:],
                                     start=True, stop=True)
                    nc.scalar.copy(a3[:, tcols], zp[:])
                    nc.vector.tensor_reduce(scol3[:, t:t + 1], a3[:, tcols],
                                            axis=mybir.AxisListType.X, op=OP.add)
                    sq = ttpool.tile([64, 128], F32, tag="sq3t")
                    nc.scalar.square(sq[:], a3[:, tcols])
                    nc.vector.tensor_reduce(qcol3[:, t:t + 1], sq[:],
                                            axis=mybir.AxisListType.X, op=OP.add)
                nc.vector.tensor_reduce(bn3_sb[:64, 0:1], scol3[:],
                                        axis=mybir.AxisListType.X, op=OP.add)
                nc.vector.tensor_reduce(bn3_sb[:64, 1:2], qcol3[:],
                                        axis=mybir.AxisListType.X, op=OP.add)
                nc.sync.dma_start(bn_in_d[2][:64, :], bn3_sb[:64, :])
                zf = tspool.tile([64, 2], F32, tag="zf3")
                nc.vector.memset(zf[:], 0.0)
                nc.sync.dma_start(bn_in_d[2][64:, :], zf[:])
                nc.gpsimd.collective_compute(
                    "AllReduce", OP.add, replica_groups=rg,
                    ins=[bn_in_d[2][:].opt()], outs=[bn_out_d[2][:].opt()])
                nc.sync.dma_start(bn3_sb[:64, :], bn_out_d[2][:64, :])
                nc.vector.tensor_tensor(out=bn3_sb[:64, :], in0=bn3_sb[:64, :],
                                        in1=zpad3_keep[:], op=OP.add)
                st = bn3st
                nc.scalar.mul(st[:, 0:1], bn3_sb[:64, 0:1], invN)
                nc.scalar.mul(st[:, 1:2], bn3_sb[:64, 1:2], invN)
                nc.vector.tensor_tensor(out=st[:, 2:3], in0=st[:, 0:1],
                                        in1=st[:, 0:1], op=OP.mult)
                nc.vector.tensor_tensor(out=st[:, 2:3], in0=st[:, 1:2],
                                        in1=st[:, 2:3], op=OP.subtract)
                nc.scalar.activation(st[:, 3:4], st[:, 2:3], AF.Sqrt,
                                     bias=epsc[:64, 0:1], scale=1.0)
                nc.vector.reciprocal(st[:, 4:5], st[:, 3:4])
                nc.vector.tensor_tensor(out=st[:, 4:5], in0=st[:, 4:5],
                                        in1=g3sb[:, 0:1], op=OP.mult)
                nc.vector.tensor_tensor(out=st[:, 5:6], in0=st[:, 0:1],
                                        in1=st[:, 4:5], op=OP.mult)
                nc.vector.tensor_tensor(out=st[:, 5:6], in0=be3sb[:, 0:1],
                                        in1=st[:, 5:6], op=OP.subtract)

                for t in range(BPC):
                    tcols = slice(t * BLK, (t + 1) * BLK)
                    nc.scalar.activation(a3h[:, tcols], a3[:, tcols], AF.Relu,
                                         bias=st[:, 5:6], scale=st[:, 4:5])
                    po_a = tpps.tile([128, OUT_A], F32, space="PSUM", tag="poa")
                    nc.tensor.matmul(out=po_a[:], lhsT=a3h[:, tcols],
                                     rhs=wosb[:, 0:OUT_A], start=True, stop=True)
                    ost = ttpool.tile([128, OUT], F32, tag="ost")
                    nc.vector.tensor_tensor(out=ost[:, 0:OUT_A], in0=po_a[:],
                                            in1=boutR[:, 0:OUT_A], op=OP.add)
                    if OUT_B:
                        po_b = tpps.tile([128, OUT_B], F32, space="PSUM",
                                         tag="pob")
                        nc.tensor.matmul(out=po_b[:], lhsT=a3h[:, tcols],
                                         rhs=wosb[:, OUT_A:OUT], start=True,
                                         stop=True)
                        nc.vector.tensor_tensor(out=ost[:, OUT_A:OUT],
                                                in0=po_b[:],
                                                in1=boutR[:, OUT_A:OUT],
                                                op=OP.add)
                    nc.sync.dma_start(out[t * BLK:(t + 1) * BLK, :], ost[:])

    nc.compile()
    return nc


# ---------------------------------------------------------------- runner

_CACHE = {}


def run(inputs: dict, cfg: Cfg, trace: bool = False):
    import ml_dtypes
    from concourse.bass_utils import run_bass_kernel_spmd

    edge_index = np.asarray(inputs["edge_index"])
    perm, idx_img, dst_img = build_edge_structures(cfg, edge_index)
    dst_img = dst_img.astype(ml_dtypes.bfloat16)
    wout_bf = np.asarray(inputs["Wout"], np.float32).astype(ml_dtypes.bfloat16)

    x = np.asarray(inputs["x"], np.float32)
    xp = np.zeros((cfg.NP, cfg.IN), np.float32)
    xp[perm[:cfg.N]] = x

    def col(v, parts):
        return np.asarray(v, np.float32).reshape(parts, 1)

    in_maps = []
    for c in range(cfg.NCORES):
        in_maps.append({
            "x_sh": xp[c * cfg.ROWS:(c + 1) * cfg.ROWS],
            "idxs": idx_img[c],
            "drel": dst_img[c],
            "W1": np.asarray(inputs["W1"], np.float32),
            "W2": np.asarray(inputs["W2"], np.float32),
            "W3": np.asarray(inputs["W3"], np.float32),
            "Wout": wout_bf,
            "g1": col(inputs["g1"], 128), "be1": col(inputs["be1"], 128),
            "g2": col(inputs["g2"], 128), "be2": col(inputs["be2"], 128),
            "g3": col(inputs["g3"], 64), "be3": col(inputs["be3"], 64),
            "att": np.asarray(inputs["att"], np.float32).reshape(1, -1),
            "bout": np.asarray(inputs["bout"], np.float32).reshape(1, -1),
        })

    key = (cfg.N, cfg.E, cfg.K, cfg.BPC, cfg.OUT)
    if key not in _CACHE:
        _CACHE[key] = build_nc(cfg)
    nc = _CACHE[key]

    res = run_bass_kernel_spmd(nc, in_maps, core_ids=list(range(cfg.NCORES)),
                               trace=trace)
    outp = np.concatenate([res.results[c]["out"] for c in range(cfg.NCORES)], 0)
    outf = outp[perm[:cfg.N]]
    return outf.astype(np.float32), res


def kernel(**inputs) -> np.ndarray:
    out, _ = run(inputs, FULL)
    return out



# revision 31
# speedup vs baseline: 2.9059x; 1.5599x over previous
"""DAGNN recommender forward pass on 8 Trainium2 NeuronCores (Bass/Tile).

Strategy (nodes sharded across cores, dst-block scatter via selection-matrix
matmuls, per-hop AllGather of the updated node states):

 - Nodes are relabeled by a host-side permutation so each 128-row "block"
   has a balanced in-edge count (<= 2048).  400 blocks total, 50 per core.
 - Per hop, each core gathers cur[src] rows for its ~100k in-edges with
   `dma_gather` (512B rows from a replicated DRAM copy of cur), then for
   every dst block accumulates 16 matmuls  psum += S_chunk.T @ G_chunk
   where S is the one-hot (edge -> dst row) selection matrix built on the
   fly with iota + is_equal.  Gather indices are int16, so gathers read
   from two 32768-row windows of cur (A = rows [0, 32768), B = last 32768
   rows); the host assigns each edge to a window.
 - softmax(att) hop weights are folded into cur (cur'_i = w_i A^i h), so
   the DAGNN output is just the running sum of cur' slices.
 - The input/output MLPs run feature-major (activations transposed) so
   BatchNorm's per-feature statistics live on partitions; train-mode BN
   makes the b1/b2/b3 biases cancel, and the zero-padded fake nodes are
   corrected analytically in the BN statistics.
 - Cross-core traffic: one AllGather of the cur slices per hop plus three
   tiny AllReduces for BN statistics.
"""

import os
import sys

if "/opt/trn_rl_repo" not in sys.path:
    sys.path.insert(0, "/opt/trn_rl_repo")

import numpy as np


# ---------------------------------------------------------------- config

class Cfg:
    def __init__(self, N=50000, E=800000, K=10, BPC=50, OUT=1000):
        self.N, self.E, self.K, self.BPC, self.OUT = N, E, K, BPC, OUT
        self.H = 128
        self.IN = 128
        self.NCORES = 8
        self.BLK = 128
        self.NBLOCKS = self.NCORES * BPC
        self.NP = self.NBLOCKS * self.BLK
        self.ROWS = BPC * self.BLK              # rows per core
        self.ACH = 8
        self.BCH = 8
        self.CH = self.ACH + self.BCH
        self.ASLOTS = self.ACH * self.BLK       # 1024
        self.BSLOTS = self.BCH * self.BLK
        self.WINW = min(32768, self.NP)
        self.WINA_LO = 0
        self.WINB_LO = self.NP - self.WINW
        self.NPADN = self.NP - N
        self.GG = 2                              # blocks per gather group
        assert BPC % self.GG == 0
        self.NGROUPS = BPC // self.GG
        self.EPS = 1e-5


FULL = Cfg()


# ---------------------------------------------------------------- host prep

def balance_nodes(cfg: Cfg, dst: np.ndarray) -> np.ndarray:
    """perm: old node id (incl. pads) -> new padded row id, balancing
    per-block in-edge counts (LPT greedy)."""
    import heapq
    deg = np.bincount(dst, minlength=cfg.N).astype(np.int64)
    deg_all = np.concatenate([deg, np.zeros(cfg.NP - cfg.N, np.int64)])
    order = np.argsort(-deg_all, kind="stable")
    load = np.zeros(cfg.NBLOCKS, np.int64)
    fill = np.zeros(cfg.NBLOCKS, np.int64)
    perm = np.empty(cfg.NP, np.int64)
    heap = [(0, b) for b in range(cfg.NBLOCKS)]
    heapq.heapify(heap)
    for node in order:
        while True:
            _, b = heapq.heappop(heap)
            if fill[b] < cfg.BLK:
                break
        perm[node] = b * cfg.BLK + fill[b]
        fill[b] += 1
        load[b] += deg_all[node]
        if fill[b] < cfg.BLK:
            heapq.heappush(heap, (load[b], b))
    return perm


def build_edge_structures(cfg: Cfg, edge_index: np.ndarray):
    """(perm, idx_img [8,128,BPC*128] int16, dst_img [8,128,BPC*16] f32)."""
    src, dst = edge_index[0].astype(np.int64), edge_index[1].astype(np.int64)
    perm = balance_nodes(cfg, dst)
    psrc = perm[src]
    pdst = perm[dst]
    blk = pdst // cfg.BLK
    rel = pdst % cfg.BLK
    order = np.argsort(blk, kind="stable")
    psrc, rel, blk = psrc[order], rel[order], blk[order]
    starts = np.searchsorted(blk, np.arange(cfg.NBLOCKS + 1))

    acols = cfg.ASLOTS // 16                    # idx cols per block per region
    idx_img = np.zeros((cfg.NCORES, 16, 2 * cfg.BPC * acols), np.int16)
    dst_img = np.full((cfg.NCORES, 128, cfg.BPC * cfg.CH), -1.0, np.float32)
    aoff = cfg.BPC * acols                      # col offset of B region

    for b in range(cfg.NBLOCKS):
        s, e = starts[b], starts[b + 1]
        bs, br = psrc[s:e], rel[s:e]
        assert e - s <= cfg.ASLOTS + cfg.BSLOTS, (b, e - s)
        mustA = bs < cfg.WINB_LO
        mustB = bs >= cfg.WINW
        flex = ~mustA & ~mustB
        nA, nB, nF = int(mustA.sum()), int(mustB.sum()), int(flex.sum())
        assert nA <= cfg.ASLOTS and nB <= cfg.BSLOTS, (b, nA, nB)
        takeA = min(cfg.ASLOTS - nA, nF)
        assert nB + (nF - takeA) <= cfg.BSLOTS, (b, nA, nB, nF)
        fi = np.nonzero(flex)[0]
        a_sel = np.concatenate([np.nonzero(mustA)[0], fi[:takeA]])
        b_sel = np.concatenate([np.nonzero(mustB)[0], fi[takeA:]])

        a_idx = np.zeros(cfg.ASLOTS, np.int64)
        a_rel = np.full(cfg.ASLOTS, -1.0, np.float32)
        a_idx[:len(a_sel)] = bs[a_sel] - cfg.WINA_LO
        a_rel[:len(a_sel)] = br[a_sel]
        b_idx = np.zeros(cfg.BSLOTS, np.int64)
        b_rel = np.full(cfg.BSLOTS, -1.0, np.float32)
        b_idx[:len(b_sel)] = bs[b_sel] - cfg.WINB_LO
        b_rel[:len(b_sel)] = br[b_sel]
        assert 0 <= a_idx.min(initial=0) and a_idx.max(initial=0) < 32768
        assert 0 <= b_idx.min(initial=0) and b_idx.max(initial=0) < 32768

        c, lb = divmod(b, cfg.BPC)
        idx_img[c, :, lb * acols:(lb + 1) * acols] = \
            a_idx.reshape(acols, 16).T.astype(np.int16)
        idx_img[c, :, aoff + lb * acols: aoff + (lb + 1) * acols] = \
            b_idx.reshape(acols, 16).T.astype(np.int16)
        dst_img[c, :, lb * cfg.ACH:(lb + 1) * cfg.ACH] = \
            a_rel.reshape(cfg.ACH, 128).T
        boff = cfg.BPC * cfg.ACH
        dst_img[c, :, boff + lb * cfg.BCH: boff + (lb + 1) * cfg.BCH] = \
            b_rel.reshape(cfg.BCH, 128).T

    idx_img = np.tile(idx_img, (1, 8, 1))       # replicate to 128 partitions
    # 16x-expanded dst image (each rel-dst value repeated 16x along cols) so
    # the on-chip is_equal runs with a packed last dim (DVE 2x_1p mode).
    dst_exp = np.repeat(dst_img, 16, axis=2)
    return perm, idx_img, dst_exp


# ---------------------------------------------------------------- device

def build_nc(cfg: Cfg):
    from concourse import bass, mybir, bacc, tile

    F32 = mybir.dt.float32
    BF16 = mybir.dt.bfloat16
    I16 = mybir.dt.int16
    I32 = mybir.dt.int32
    AF = mybir.ActivationFunctionType
    OP = mybir.AluOpType
    BLK, BPC, CH, ACH, BCH = cfg.BLK, cfg.BPC, cfg.CH, cfg.ACH, cfg.BCH
    ROWS, NP, K, H, OUT = cfg.ROWS, cfg.NP, cfg.K, cfg.H, cfg.OUT
    GG, NG = cfg.GG, cfg.NGROUPS
    acols = cfg.ASLOTS // 16
    NCORES = cfg.NCORES
    invN = 1.0 / cfg.N
    OUT_A = min(512, OUT)
    OUT_B = OUT - OUT_A

    nc = bacc.Bacc("TRN2", target_bir_lowering=False, debug=False,
                   num_devices=NCORES, num_swdge_queues=4)

    x_sh = nc.dram_tensor("x_sh", [ROWS, cfg.IN], F32, kind="ExternalInput")
    idxs = nc.dram_tensor("idxs", [128, 2 * BPC * acols], I16, kind="ExternalInput")
    drel = nc.dram_tensor("drel", [128, BPC * CH * 16], BF16, kind="ExternalInput")
    W1 = nc.dram_tensor("W1", [cfg.IN, H], F32, kind="ExternalInput")
    W2 = nc.dram_tensor("W2", [H, H], F32, kind="ExternalInput")
    W3 = nc.dram_tensor("W3", [H, H // 2], F32, kind="ExternalInput")
    Wout = nc.dram_tensor("Wout", [H // 2, OUT], BF16, kind="ExternalInput")
    g1 = nc.dram_tensor("g1", [H, 1], F32, kind="ExternalInput")
    be1 = nc.dram_tensor("be1", [H, 1], F32, kind="ExternalInput")
    g2 = nc.dram_tensor("g2", [H, 1], F32, kind="ExternalInput")
    be2 = nc.dram_tensor("be2", [H, 1], F32, kind="ExternalInput")
    g3 = nc.dram_tensor("g3", [H // 2, 1], F32, kind="ExternalInput")
    be3 = nc.dram_tensor("be3", [H // 2, 1], F32, kind="ExternalInput")
    att = nc.dram_tensor("att", [1, K + 1], F32, kind="ExternalInput")
    bout = nc.dram_tensor("bout", [1, OUT], F32, kind="ExternalInput")
    out = nc.dram_tensor("out", [ROWS, OUT], F32, kind="ExternalOutput")

    rg = [list(range(NCORES))]

    with tile.TileContext(nc) as tc:
        with (
            tc.tile_pool(name="const", bufs=1) as cpool,
            tc.tile_pool(name="resid", bufs=1) as rpool,
            tc.tile_pool(name="dram", bufs=1, space="DRAM") as dpool,
        ):
            # ---------------- persistent DRAM buffers
            # One AllGather output buffer per hop (K+1 total).
            curbufs = [dpool.tile([NP, H], BF16, tag=f"cur{i}",
                                  name=f"cur{i}")
                       for i in range(K + 1)]
            ag_in = dpool.tile([ROWS, H], BF16, tag="ag_in")
            bn_in_d = [dpool.tile([H, 2], F32, tag=f"bni{i}", name=f"bni{i}") for i in range(3)]
            bn_out_d = [dpool.tile([H, 2], F32, tag=f"bno{i}", name=f"bno{i}") for i in range(3)]

            # ---------------- constants / weights to SBUF
            w1sb = cpool.tile([cfg.IN, H], F32)
            nc.sync.dma_start(w1sb[:], W1[:])
            w2sb = cpool.tile([H, H], F32)
            nc.sync.dma_start(w2sb[:], W2[:])
            w3sb = cpool.tile([H, H // 2], F32)
            nc.sync.dma_start(w3sb[:], W3[:])
            wosb = cpool.tile([H // 2, OUT], BF16)
            nc.sync.dma_start(wosb[:], Wout[:])
            g1sb = cpool.tile([H, 1], F32); nc.sync.dma_start(g1sb[:], g1[:])
            be1sb = cpool.tile([H, 1], F32); nc.sync.dma_start(be1sb[:], be1[:])
            g2sb = cpool.tile([H, 1], F32); nc.sync.dma_start(g2sb[:], g2[:])
            be2sb = cpool.tile([H, 1], F32); nc.sync.dma_start(be2sb[:], be2[:])
            g3sb = cpool.tile([H // 2, 1], F32); nc.sync.dma_start(g3sb[:], g3[:])
            be3sb = cpool.tile([H // 2, 1], F32); nc.sync.dma_start(be3sb[:], be3[:])
            attsb = cpool.tile([1, K + 1], F32); nc.sync.dma_start(attsb[:], att[:])
            bosb = cpool.tile([1, OUT], F32); nc.sync.dma_start(bosb[:], bout[:])

            idx_sb = cpool.tile([128, 2 * BPC * acols], I16)
            nc.sync.dma_start(idx_sb[:], idxs[:])
            dstT = cpool.tile([128, BPC * CH * 16], BF16)
            nc.sync.dma_start(dstT[:], drel[:])

            identity = cpool.tile([128, 128], F32)
            from concourse.masks import make_identity
            make_identity(nc, identity[:])

            iota_i = cpool.tile([128, ACH * 128], I32)
            nc.gpsimd.iota(iota_i[:].rearrange("p (c d) -> p c d", d=128),
                           pattern=[[0, ACH], [1, 128]], base=0,
                           channel_multiplier=0)
            iota8f = cpool.tile([128, ACH * 128], F32)
            nc.vector.tensor_copy(iota8f[:], iota_i[:])
            iota_bf = cpool.tile([128, ACH * 128], BF16)
            nc.vector.tensor_copy(iota_bf[:], iota8f[:])

            ones1 = cpool.tile([1, 128], F32)
            nc.vector.memset(ones1[:], 1.0)
            epsc = cpool.tile([128, 1], F32)
            nc.vector.memset(epsc[:], cfg.EPS)

            # softmax(att) -> w[0..K], ratios r_i = w_i/w_{i-1} -> wR [128, 2K+1]
            mx = cpool.tile([1, 1], F32)
            nc.vector.tensor_reduce(mx[:], attsb[:], axis=mybir.AxisListType.X,
                                    op=OP.max)
            nmx = cpool.tile([1, 1], F32)
            nc.scalar.mul(nmx[:], mx[:], -1.0)
            ew = cpool.tile([1, K + 1], F32)
            nc.scalar.activation(ew[:], attsb[:], AF.Exp, bias=nmx[:, 0:1],
                                 scale=1.0)
            ssum = cpool.tile([1, 1], F32)
            nc.vector.tensor_reduce(ssum[:], ew[:], axis=mybir.AxisListType.X,
                                    op=OP.add)
            rsum = cpool.tile([1, 1], F32)
            nc.vector.reciprocal(rsum[:], ssum[:])
            wv = cpool.tile([1, K + 1], F32)
            nc.vector.tensor_scalar_mul(wv[:], ew[:], rsum[:, 0:1])
            rw = cpool.tile([1, K + 1], F32)
            nc.vector.reciprocal(rw[:], wv[:])
            wcat = cpool.tile([1, 2 * K + 1], F32)
            nc.vector.tensor_copy(wcat[:, 0:K + 1], wv[:])
            nc.vector.tensor_tensor(out=wcat[:, K + 1:2 * K + 1],
                                    in0=wv[:, 1:K + 1], in1=rw[:, 0:K],
                                    op=OP.mult)
            with tc.tile_pool(name="wps", bufs=1, space="PSUM") as wps:
                wpsum = wps.tile([128, 2 * K + 1], F32, space="PSUM")
                nc.tensor.matmul(out=wpsum[:], lhsT=ones1[:], rhs=wcat[:],
                                 start=True, stop=True)
                wR = cpool.tile([128, 2 * K + 1], F32)
                nc.scalar.copy(wR[:], wpsum[:])

                # bout replicated to 128 partitions
                boutR = cpool.tile([128, OUT], F32)
                bps_a = wps.tile([128, OUT_A], F32, space="PSUM", tag="bps")
                nc.tensor.matmul(out=bps_a[:], lhsT=ones1[:],
                                 rhs=bosb[:, 0:OUT_A], start=True, stop=True)
                nc.scalar.copy(boutR[:, 0:OUT_A], bps_a[:])
                if OUT_B:
                    bps_b = wps.tile([128, OUT_B], F32, space="PSUM", tag="bps")
                    nc.tensor.matmul(out=bps_b[:], lhsT=ones1[:],
                                     rhs=bosb[:, OUT_A:OUT], start=True,
                                     stop=True)
                    nc.scalar.copy(boutR[:, OUT_A:OUT], bps_b[:])

            # ---------------- resident activations
            stage = rpool.tile([128, ROWS], BF16, tag="stage")   # node-major cur'
            accum = rpool.tile([128, ROWS], F32, tag="accum")    # sum of cur'

            # ================ input MLP (feature-major) ================
            with (
                tc.tile_pool(name="mlp_a", bufs=1) as apool,
                tc.tile_pool(name="mlp_t", bufs=4) as tpool,
                tc.tile_pool(name="mlp_ps", bufs=3, space="PSUM") as mpps,
                tc.tile_pool(name="stat", bufs=1) as spool,
            ):
                a1 = apool.tile([128, ROWS], F32, tag="a1")      # z1T then h1T
                a2 = apool.tile([128, ROWS], F32, tag="a2")      # z2T then h2T
                scol = spool.tile([128, BPC], F32, tag="scol")
                qcol = spool.tile([128, BPC], F32, tag="qcol")
                bn_sb = [spool.tile([128, 2], F32, tag=f"bnsb{i}", name=f"bnsb{i}")
                         for i in range(3)]
                bnst = [spool.tile([128, 6], F32, tag=f"bnst{i}", name=f"bnst{i}")
                        for i in range(3)]  # mean, ex2, var, std, scale, shift

                def bn_stats_finish(i, gsb, besb, parts=128):
                    """bn_sb[i][:,0]=sum, [:,1]=sumsq (already AllReduced &
                    pad-corrected) -> bnst[i] cols: scale(4), shift(5)."""
                    st = bnst[i]
                    nc.scalar.mul(st[:parts, 0:1], bn_sb[i][:parts, 0:1], invN)
                    nc.scalar.mul(st[:parts, 1:2], bn_sb[i][:parts, 1:2], invN)
                    nc.vector.tensor_tensor(out=st[:parts, 2:3],
                                            in0=st[:parts, 0:1],
                                            in1=st[:parts, 0:1], op=OP.mult)
                    nc.vector.tensor_tensor(out=st[:parts, 2:3],
                                            in0=st[:parts, 1:2],
                                            in1=st[:parts, 2:3], op=OP.subtract)
                    nc.scalar.activation(st[:parts, 3:4], st[:parts, 2:3],
                                         AF.Sqrt, bias=epsc[:parts, 0:1],
                                         scale=1.0)
                    nc.vector.reciprocal(st[:parts, 4:5], st[:parts, 3:4])
                    nc.vector.tensor_tensor(out=st[:parts, 4:5],
                                            in0=st[:parts, 4:5],
                                            in1=gsb[:parts, 0:1], op=OP.mult)
                    nc.vector.tensor_tensor(out=st[:parts, 5:6],
                                            in0=st[:parts, 0:1],
                                            in1=st[:parts, 4:5], op=OP.mult)
                    nc.vector.tensor_tensor(out=st[:parts, 5:6],
                                            in0=besb[:parts, 0:1],
                                            in1=st[:parts, 5:6], op=OP.subtract)

                def bn_allreduce(i, parts=128):
                    nc.sync.dma_start(bn_in_d[i][:parts, :], bn_sb[i][:parts, :])
                    if parts < 128:
                        zf = spool.tile([128 - parts, 2], F32, tag="zfill")
                        nc.vector.memset(zf[:], 0.0)
                        nc.sync.dma_start(bn_in_d[i][parts:, :], zf[:])
                    nc.gpsimd.collective_compute(
                        "AllReduce", OP.add, replica_groups=rg,
                        ins=[bn_in_d[i][:].opt()], outs=[bn_out_d[i][:].opt()])
                    nc.sync.dma_start(bn_sb[i][:parts, :], bn_out_d[i][:parts, :])

                # ---- MLP1: z1T = W1.T @ xT
                for t in range(BPC):
                    xe = tpool.tile([128, 128], F32, tag="xload")
                    nc.sync.dma_start(xe[:], x_sh[t * BLK:(t + 1) * BLK, :])
                    xtp = mpps.tile([128, 128], F32, space="PSUM", tag="xtp")
                    nc.tensor.transpose(xtp[:], xe[:], identity[:])
                    xt = tpool.tile([128, 128], F32, tag="xt")
                    nc.scalar.copy(xt[:], xtp[:])
                    zp = mpps.tile([128, 128], F32, space="PSUM", tag="zp")
                    nc.tensor.matmul(out=zp[:], lhsT=w1sb[:], rhs=xt[:],
                                     start=True, stop=True)
                    tcols = slice(t * BLK, (t + 1) * BLK)
                    nc.scalar.copy(a1[:, tcols], zp[:])
                    nc.vector.tensor_reduce(scol[:, t:t + 1], a1[:, tcols],
                                            axis=mybir.AxisListType.X, op=OP.add)
                    sq = tpool.tile([128, 128], F32, tag="sq")
                    nc.scalar.square(sq[:], a1[:, tcols])
                    nc.vector.tensor_reduce(qcol[:, t:t + 1], sq[:],
                                            axis=mybir.AxisListType.X, op=OP.add)
                nc.vector.tensor_reduce(bn_sb[0][:, 0:1], scol[:],
                                        axis=mybir.AxisListType.X, op=OP.add)
                nc.vector.tensor_reduce(bn_sb[0][:, 1:2], qcol[:],
                                        axis=mybir.AxisListType.X, op=OP.add)
                bn_allreduce(0)
                bn_stats_finish(0, g1sb, be1sb)
                sc1 = bnst[0][:, 4:5]
                sh1 = bnst[0][:, 5:6]
                # h1T = relu(sc1 * z1T + sh1), in place
                for t in range(BPC):
                    tcols = slice(t * BLK, (t + 1) * BLK)
                    nc.scalar.activation(a1[:, tcols], a1[:, tcols], AF.Relu,
                                         bias=sh1, scale=sc1)

                # pad row value hpad1 = relu(sh1)
                hpad1 = spool.tile([128, 1], F32, tag="hpad1")
                nc.scalar.activation(hpad1[:], sh1, AF.Relu)
                zpad2 = spool.tile([128, 1], F32, tag="zpad2")
                zp2p = mpps.tile([128, 1], F32, space="PSUM", tag="zp")
                nc.tensor.matmul(out=zp2p[:], lhsT=w2sb[:], rhs=hpad1[:],
                                 start=True, stop=True)
                nc.scalar.copy(zpad2[:], zp2p[:])

                # ---- MLP2: z2T = W2.T @ h1T
                for t in range(BPC):
                    tcols = slice(t * BLK, (t + 1) * BLK)
                    zp = mpps.tile([128, 128], F32, space="PSUM", tag="zp")
                    nc.tensor.matmul(out=zp[:], lhsT=w2sb[:], rhs=a1[:, tcols],
                                     start=True, stop=True)
                    nc.scalar.copy(a2[:, tcols], zp[:])
                    nc.vector.tensor_reduce(scol[:, t:t + 1], a2[:, tcols],
                                            axis=mybir.AxisListType.X, op=OP.add)
                    sq = tpool.tile([128, 128], F32, tag="sq")
                    nc.scalar.square(sq[:], a2[:, tcols])
                    nc.vector.tensor_reduce(qcol[:, t:t + 1], sq[:],
                                            axis=mybir.AxisListType.X, op=OP.add)
                nc.vector.tensor_reduce(bn_sb[1][:, 0:1], scol[:],
                                        axis=mybir.AxisListType.X, op=OP.add)
                nc.vector.tensor_reduce(bn_sb[1][:, 1:2], qcol[:],
                                        axis=mybir.AxisListType.X, op=OP.add)
                # pad correction (only core 0 contributes it, pre-AllReduce)
                # every core computes sum over its own rows; pads exist on all
                # cores' slices, already included -> subtract npad * zpad2
                # AFTER AllReduce.  Simpler: subtract (npad/NCORES-weighted)?
                # pads per core vary; correct GLOBALLY after the AllReduce.
                bn_allreduce(1)
                corr = spool.tile([128, 2], F32, tag="corr")
                nc.scalar.mul(corr[:, 0:1], zpad2[:], -float(cfg.NPADN))
                sqz = spool.tile([128, 1], F32, tag="sqz")
                nc.scalar.square(sqz[:], zpad2[:])
                nc.scalar.mul(corr[:, 1:2], sqz[:], -float(cfg.NPADN))
                nc.vector.tensor_tensor(out=bn_sb[1][:], in0=bn_sb[1][:],
                                        in1=corr[:], op=OP.add)
                bn_stats_finish(1, g2sb, be2sb)
                sc2 = bnst[1][:, 4:5]
                sh2 = bnst[1][:, 5:6]
                # h2T = relu(sc2*z2T + sh2) + h1T   (a2 in place)
                for t in range(BPC):
                    tcols = slice(t * BLK, (t + 1) * BLK)
                    nc.scalar.activation(a2[:, tcols], a2[:, tcols], AF.Relu,
                                         bias=sh2, scale=sc2)
                    nc.vector.tensor_tensor(out=a2[:, tcols], in0=a2[:, tcols],
                                            in1=a1[:, tcols], op=OP.add)

                # hpad2 = relu(sc2*zpad2+sh2) + hpad1 ; zpad3 = (w0*hpad2)@W3
                hpad2 = spool.tile([128, 1], F32, tag="hpad2")
                nc.scalar.activation(hpad2[:], zpad2[:], AF.Relu, bias=sh2,
                                     scale=sc2)
                nc.vector.tensor_tensor(out=hpad2[:], in0=hpad2[:],
                                        in1=hpad1[:], op=OP.add)
                w0hpad2 = spool.tile([128, 1], F32, tag="w0hpad2")
                nc.scalar.activation(w0hpad2[:], hpad2[:], AF.Copy, bias=0.0,
                                     scale=wR[:, 0:1])
                zpad3 = spool.tile([64, 1], F32, tag="zpad3")
                zp3p = mpps.tile([64, 1], F32, space="PSUM", tag="zp")
                nc.tensor.matmul(out=zp3p[:], lhsT=w3sb[:], rhs=w0hpad2[:],
                                 start=True, stop=True)
                nc.scalar.copy(zpad3[:], zp3p[:])
                zpad3_keep = rpool.tile([64, 2], F32, tag="zpad3k")
                nc.scalar.mul(zpad3_keep[:, 0:1], zpad3[:], -float(cfg.NPADN))
                sq3 = spool.tile([64, 1], F32, tag="sq3")
                nc.scalar.square(sq3[:], zpad3[:])
                nc.scalar.mul(zpad3_keep[:, 1:2], sq3[:], -float(cfg.NPADN))

                # stage = h2 node-major; accum = w0 * stage
                for t in range(BPC):
                    tcols = slice(t * BLK, (t + 1) * BLK)
                    htp = mpps.tile([128, 128], F32, space="PSUM", tag="xtp")
                    nc.tensor.transpose(htp[:], a2[:, tcols], identity[:])
                    nc.scalar.copy(stage[:, tcols], htp[:])
                    nc.scalar.activation(accum[:, tcols], htp[:],
                                         AF.Copy, bias=0.0, scale=wR[:, 0:1])

            # write cur'_0 and AllGather it
            nc.sync.dma_start(
                ag_in[:].rearrange("(b p) f -> p b f", p=BLK),
                stage[:].rearrange("p (b f) -> p b f", f=H))
            nc.gpsimd.collective_compute(
                "AllGather", mybir.AluOpType.bypass, replica_groups=rg,
                ins=[ag_in[:].opt()], outs=[curbufs[0][:].opt()])

            # ================ K propagation hops ================
            KPH = os.environ.get("DAGNN_KPH", "full")
            NHOP = 0 if KPH == "noprop" else K
            with (
                tc.tile_pool(name="gat", bufs=4) as gpool,
                tc.tile_pool(name="spool", bufs=4) as sbpool,
                tc.tile_pool(name="hps", bufs=4, space="PSUM") as hpps,
            ):
                asl = GG * cfg.ASLOTS
                for i in range(1, NHOP + 1):
                    rd = curbufs[i - 1]
                    wr = curbufs[i]
                    rcol = wR[:, K + i: K + i + 1]        # w_i / w_{i-1}
                    for g in range(NG):
                        gt = gpool.tile([128, GG * CH * 128], BF16, tag="gt")
                        nc.gpsimd.dma_gather(
                            gt[:, 0:asl].rearrange("p (c f) -> p c f", f=128),
                            rd[cfg.WINA_LO:cfg.WINA_LO + cfg.WINW, :],
                            idx_sb[:, g * GG * acols:(g + 1) * GG * acols],
                            asl, asl, 128, single_packet=False,
                            queue_num=(2 * g) % 4)
                        nc.gpsimd.dma_gather(
                            gt[:, asl:2 * asl].rearrange("p (c f) -> p c f",
                                                         f=128),
                            rd[cfg.WINB_LO:cfg.WINB_LO + cfg.WINW, :],
                            idx_sb[:, (BPC + g * GG) * acols:
                                   (BPC + (g + 1) * GG) * acols],
                            asl, asl, 128, single_packet=False,
                            queue_num=(2 * g + 1) % 4)
                        for j in range(GG):
                            b = g * GG + j
                            sA = sbpool.tile([128, ACH * 128], BF16, tag="sA")
                            nc.vector.tensor_tensor(
                                out=sA[:].rearrange("p (c e r) -> p c e r",
                                                    e=8, r=16),
                                in0=iota_bf[:].rearrange("p (c e r) -> p c e r",
                                                         e=8, r=16),
                                in1=dstT[:, b * ACH * 16:(b + 1) * ACH * 16]
                                    .rearrange("p (c r) -> p c r", r=16)
                                    .unsqueeze(2)
                                    .to_broadcast([128, ACH, 8, 16]),
                                op=OP.is_equal)
                            sB = sbpool.tile([128, BCH * 128], BF16, tag="sB")
                            boff = BPC * ACH * 16
                            nc.vector.tensor_tensor(
                                out=sB[:].rearrange("p (c e r) -> p c e r",
                                                    e=8, r=16),
                                in0=iota_bf[:].rearrange("p (c e r) -> p c e r",
                                                         e=8, r=16),
                                in1=dstT[:, boff + b * BCH * 16:
                                         boff + (b + 1) * BCH * 16]
                                    .rearrange("p (c r) -> p c r", r=16)
                                    .unsqueeze(2)
                                    .to_broadcast([128, BCH, 8, 16]),
                                op=OP.is_equal)
                            ps = hpps.tile([128, 128], F32, space="PSUM",
                                           tag="hps")
                            for k in range(ACH):
                                nc.tensor.matmul(
                                    out=ps[:],
                                    lhsT=sA[:, k * 128:(k + 1) * 128],
                                    rhs=gt[:, (j * ACH + k) * 128:
                                           (j * ACH + k + 1) * 128],
                                    start=(k == 0), stop=False)
                            for k in range(BCH):
                                nc.tensor.matmul(
                                    out=ps[:],
                                    lhsT=sB[:, k * 128:(k + 1) * 128],
                                    rhs=gt[:, asl + (j * BCH + k) * 128:
                                           asl + (j * BCH + k + 1) * 128],
                                    start=False, stop=(k == BCH - 1))
                            tcols = slice(b * BLK, (b + 1) * BLK)
                            nc.scalar.activation(stage[:, tcols], ps[:],
                                                 AF.Copy, bias=0.0, scale=rcol)
                            # accumulate the unrounded f32 value: bf16-rounding
                            # the addend couples into the propagation chain and
                            # triples the final error.
                            tmp32 = sbpool.tile([128, BLK], F32, tag="t32")
                            nc.scalar.activation(tmp32[:], ps[:],
                                                 AF.Copy, bias=0.0, scale=rcol)
                            nc.vector.tensor_tensor(out=accum[:, tcols],
                                                    in0=accum[:, tcols],
                                                    in1=tmp32[:],
                                                    op=OP.add)
                    if i < NHOP:
                        nc.sync.dma_start(
                            ag_in[:].rearrange("(b p) f -> p b f", p=BLK),
                            stage[:].rearrange("p (b f) -> p b f", f=H))
                        if KPH == "noag":
                            nc.sync.dma_start(wr[0:ROWS, :], ag_in[:])
                        else:
                            nc.gpsimd.collective_compute(
                                "AllGather", mybir.AluOpType.bypass,
                                replica_groups=rg,
                                ins=[ag_in[:].opt()], outs=[wr[:].opt()])

            # ================ output MLP ================
            with (
                tc.tile_pool(name="tail_a", bufs=1) as tapool,
                tc.tile_pool(name="tail_t", bufs=4) as ttpool,
                tc.tile_pool(name="tail_ps", bufs=2, space="PSUM") as tpps,
                tc.tile_pool(name="tstat", bufs=1) as tspool,
            ):
                a3 = tapool.tile([64, ROWS], F32, tag="a3")
                a3h = tapool.tile([64, ROWS], BF16, tag="a3h")
                scol3 = tspool.tile([64, BPC], F32, tag="scol3")
                qcol3 = tspool.tile([64, BPC], F32, tag="qcol3")
                bn3_sb = tspool.tile([128, 2], F32, tag="bn3sb")
                bn3st = tspool.tile([64, 6], F32, tag="bn3st")

                for t in range(BPC):
                    tcols = slice(t * BLK, (t + 1) * BLK)
                    otp = tpps.tile([128, 128], F32, space="PSUM", tag="otp")
                    nc.tensor.transpose(otp[:], accum[:, tcols], identity[:])
                    ot = ttpool.tile([128, 128], F32, tag="ot")
                    nc.scalar.copy(ot[:], otp[:])
                    zp = tpps.tile([64, 128], F32, space="PSUM", tag="zp3")
                    nc.tensor.matmul(out=zp[:], lhsT=w3sb[:], rhs=ot[:],
                                     start=True, stop=True)
                    nc.scalar.copy(a3[:, tcols], zp[:])
                    nc.vector.tensor_reduce(scol3[:, t:t + 1], a3[:, tcols],
                                            axis=mybir.AxisListType.X, op=OP.add)
                    sq = ttpool.tile([64, 128], F32, tag="sq3t")
                    nc.scalar.square(sq[:], a3[:, tcols])
                    nc.vector.tensor_reduce(qcol3[:, t:t + 1], sq[:],
                                            axis=mybir.AxisListType.X, op=OP.add)
                nc.vector.tensor_reduce(bn3_sb[:64, 0:1], scol3[:],
                                        axis=mybir.AxisListType.X, op=OP.add)
                nc.vector.tensor_reduce(bn3_sb[:64, 1:2], qcol3[:],
                                        axis=mybir.AxisListType.X, op=OP.add)
                nc.sync.dma_start(bn_in_d[2][:64, :], bn3_sb[:64, :])
                zf = tspool.tile([64, 2], F32, tag="zf3")
                nc.vector.memset(zf[:], 0.0)
                nc.sync.dma_start(bn_in_d[2][64:, :], zf[:])
                nc.gpsimd.collective_compute(
                    "AllReduce", OP.add, replica_groups=rg,
                    ins=[bn_in_d[2][:].opt()], outs=[bn_out_d[2][:].opt()])
                nc.sync.dma_start(bn3_sb[:64, :], bn_out_d[2][:64, :])
                nc.vector.tensor_tensor(out=bn3_sb[:64, :], in0=bn3_sb[:64, :],
                                        in1=zpad3_keep[:], op=OP.add)
                st = bn3st
                nc.scalar.mul(st[:, 0:1], bn3_sb[:64, 0:1], invN)
                nc.scalar.mul(st[:, 1:2], bn3_sb[:64, 1:2], invN)
                nc.vector.tensor_tensor(out=st[:, 2:3], in0=st[:, 0:1],
                                        in1=st[:, 0:1], op=OP.mult)
                nc.vector.tensor_tensor(out=st[:, 2:3], in0=st[:, 1:2],
                                        in1=st[:, 2:3], op=OP.subtract)
                nc.scalar.activation(st[:, 3:4], st[:, 2:3], AF.Sqrt,
                                     bias=epsc[:64, 0:1], scale=1.0)
                nc.vector.reciprocal(st[:, 4:5], st[:, 3:4])
                nc.vector.tensor_tensor(out=st[:, 4:5], in0=st[:, 4:5],
                                        in1=g3sb[:, 0:1], op=OP.mult)
                nc.vector.tensor_tensor(out=st[:, 5:6], in0=st[:, 0:1],
                                        in1=st[:, 4:5], op=OP.mult)
                nc.vector.tensor_tensor(out=st[:, 5:6], in0=be3sb[:, 0:1],
                                        in1=st[:, 5:6], op=OP.subtract)

                for t in range(BPC):
                    tcols = slice(t * BLK, (t + 1) * BLK)
                    nc.scalar.activation(a3h[:, tcols], a3[:, tcols], AF.Relu,
                                         bias=st[:, 5:6], scale=st[:, 4:5])
                    po_a = tpps.tile([128, OUT_A], F32, space="PSUM", tag="poa")
                    nc.tensor.matmul(out=po_a[:], lhsT=a3h[:, tcols],
                                     rhs=wosb[:, 0:OUT_A], start=True, stop=True)
                    ost = ttpool.tile([128, OUT], F32, tag="ost")
                    nc.vector.tensor_tensor(out=ost[:, 0:OUT_A], in0=po_a[:],
                                            in1=boutR[:, 0:OUT_A], op=OP.add)
                    if OUT_B:
                        po_b = tpps.tile([128, OUT_B], F32, space="PSUM",
                                         tag="pob")
                        nc.tensor.matmul(out=po_b[:], lhsT=a3h[:, tcols],
                                         rhs=wosb[:, OUT_A:OUT], start=True,
                                         stop=True)
                        nc.vector.tensor_tensor(out=ost[:, OUT_A:OUT],
                                                in0=po_b[:],
                                                in1=boutR[:, OUT_A:OUT],
                                                op=OP.add)
                    nc.sync.dma_start(out[t * BLK:(t + 1) * BLK, :], ost[:])

    nc.compile()
    return nc


# ---------------------------------------------------------------- runner

_CACHE = {}


def run(inputs: dict, cfg: Cfg, trace: bool = False):
    import ml_dtypes
    from concourse.bass_utils import run_bass_kernel_spmd

    edge_index = np.asarray(inputs["edge_index"])
    perm, idx_img, dst_img = build_edge_structures(cfg, edge_index)
    dst_img = dst_img.astype(ml_dtypes.bfloat16)
    wout_bf = np.asarray(inputs["Wout"], np.float32).astype(ml_dtypes.bfloat16)

    x = np.asarray(inputs["x"], np.float32)
    xp = np.zeros((cfg.NP, cfg.IN), np.float32)
    xp[perm[:cfg.N]] = x

    def col(v, parts):
        return np.asarray(v, np.float32).reshape(parts, 1)

    in_maps = []
    for c in range(cfg.NCORES):
        in_maps.append({
            "x_sh": xp[c * cfg.ROWS:(c + 1) * cfg.ROWS],
            "idxs": idx_img[c],
            "drel": dst_img[c],
            "W1": np.asarray(inputs["W1"], np.float32),
            "W2": np.asarray(inputs["W2"], np.float32),
            "W3": np.asarray(inputs["W3"], np.float32),
            "Wout": wout_bf,
            "g1": col(inputs["g1"], 128), "be1": col(inputs["be1"], 128),
            "g2": col(inputs["g2"], 128), "be2": col(inputs["be2"], 128),
            "g3": col(inputs["g3"], 64), "be3": col(inputs["be3"], 64),
            "att": np.asarray(inputs["att"], np.float32).reshape(1, -1),
            "bout": np.asarray(inputs["bout"], np.float32).reshape(1, -1),
        })

    key = (cfg.N, cfg.E, cfg.K, cfg.BPC, cfg.OUT)
    if key not in _CACHE:
        _CACHE[key] = build_nc(cfg)
    nc = _CACHE[key]

    res = run_bass_kernel_spmd(nc, in_maps, core_ids=list(range(cfg.NCORES)),
                               trace=trace)
    outp = np.concatenate([res.results[c]["out"] for c in range(cfg.NCORES)], 0)
    outf = outp[perm[:cfg.N]]
    return outf.astype(np.float32), res


def kernel(**inputs) -> np.ndarray:
    out, _ = run(inputs, FULL)
    return out

